# revision 29
# baseline (speedup 1.0000x reference)
"""Trainium2 Bass kernel for nn_CAWN2 (CAWN-style GNN message passing).

Reference computation (per full input):
  seq = GRUCell(ngh_feat, hidden)                      # [B*2048, 128]
  grouped 2-head attention: q from src, k/v from seq,
  64 neighbors per (b, s) group, additive -1e10 mask,
  softmax, out proj, residual + LayerNorm, 2-layer MLP  -> [B, 32, 128]

Strategy: data-parallel over batch across 8 NeuronCores (32 batches/core).
Per core, a feature-major bf16 pipeline processes 16 supertiles of 4096
neighbor rows (2 batches):
  - PE transposes x into [d, n] tiles; copies cast to bf16 on DVE/ACT
  - fast path (hidden==0, biases==0) with tanh-only gates:
      seq' = (tanh(gz/2) - 1) * tanh(gn) = -2 * sigmoid(-gz)*tanh(gn)
    the -0.5 factor is folded into w_q and w_vs host-side, so the whole
    kernel uses one ACT table set (exp_and_others) -- no table switches
  - w_ks folded into q; scores via column-tiled matmuls with [128,32]
    stationaries (4 PE column groups), host-built additive bf16 mask
  - exp with accum_out row sums; 1/sum normalization on bf16 at 4x DVE
  - attn@v as v_j^T @ attn_j^T accumulating the output transpose directly
  - residual + LayerNorm + merge MLP batched over all 16 supertiles in
    two 512-row segments; rstd via ACT Rsqrt (one table switch at the end)
"""

import os
import sys
from contextlib import ExitStack

import numpy as np

sys.path.insert(0, "/opt/trn_rl_repo")

import ml_dtypes  # noqa: E402

import concourse.bass as bass  # noqa: E402
import concourse.bacc as bacc  # noqa: E402
import concourse.mybir as mybir  # noqa: E402
import concourse.tile as tile  # noqa: E402

F32 = mybir.dt.float32
F32R = mybir.dt.float32r
BF16 = mybir.dt.bfloat16
I32 = mybir.dt.int32
AF = mybir.ActivationFunctionType
ALU = mybir.AluOpType
AX = mybir.AxisListType

N_CORES = 8
B, N_SRC, N_NGH, D, H = 256, 32, 2048, 128, 2
DK = D // H
NN = N_NGH // N_SRC  # 64 neighbors per group
NEG_INF = -1e10
LN_EPS = 1e-5
TEMP = float(np.sqrt(DK))  # 8.0

B_CORE = B // N_CORES          # 32 batches per core
ROWS = B_CORE * N_NGH          # 65536 neighbor rows per core
SRC_ROWS = B_CORE * N_SRC      # 1024 src rows per core
ST_ROWS = 4096                 # supertile = 2 batches
N_ST = ROWS // ST_ROWS         # 16
N_GC = 8                       # 512-row GRU chunks per supertile
N_AC = 4                       # 1024-row attention chunks per supertile

_PROG_CACHE: dict = {}


def build_program_fast(tune: dict | None = None):
    """bf16 fast-path program (hidden==0, gru biases==0)."""
    tn = {"seq": 3, "x": 3, "big": 3, "med": 3, "sm": 2, "gp_stt": 1,
          "xtd_mod": 2, "pe_mask": 1, "gru1024": 1}
    tn.update(tune or {})
    nc = bacc.Bacc("TRN2")

    # ---- DRAM I/O ----
    t_ngh = nc.dram_tensor("ngh", [ROWS, D], F32, kind="ExternalInput")
    t_src = nc.dram_tensor("srcf", [SRC_ROWS, D], F32, kind="ExternalInput")
    t_mask = nc.dram_tensor("maskfull", [N_ST, 128, 1024], BF16, kind="ExternalInput")
    t_eye = nc.dram_tensor("eye", [128, 128], F32, kind="ExternalInput")
    t_eyebf = nc.dram_tensor("eyebf", [128, 128], BF16, kind="ExternalInput")
    wnames = ["wqT", "wks", "wihzT", "wihnT",
              "fcw0T", "fcw1T", "m1aT", "m1bT", "m2T"]
    t_w = {n: nc.dram_tensor(n, [128, 128], BF16, kind="ExternalInput") for n in wnames}
    t_ones2 = nc.dram_tensor("ones2", [128, 2], BF16, kind="ExternalInput")
    t_onesr = nc.dram_tensor("onesrow", [1, 128], BF16, kind="ExternalInput")
    vnames = ["fc_b", "ln_g", "ln_b", "m1b", "m2b"]
    t_v = {n: nc.dram_tensor(n, [128, 1], F32, kind="ExternalInput") for n in vnames}
    t_out = nc.dram_tensor("z", [SRC_ROWS, D], F32, kind="ExternalOutput")

    # supertile rows laid out so each DMA partition reads 4 contiguous 4KB
    # segments: supertile position (p, t) holds DRAM row
    #   d = (t//8)*1024 + (p//8)*64 + (p%8)*8 + (t%8)
    # which keeps each neighbor group inside one score ac-block; the host
    # permutes mask columns to match.
    ngh_v = t_ngh[:, :].rearrange("(st ac k i j) d -> st (k i) ac j d",
                                  st=N_ST, ac=4, k=16, i=8, j=8)
    src_v = t_src[:, :].rearrange("(t p) d -> p t d", t=8, p=128)
    out_v = t_out[:, :].rearrange("(t p) d -> p t d", t=8, p=128)

    with tile.TileContext(nc) as tc, ExitStack() as ctx:
        consts = ctx.enter_context(tc.tile_pool(name="consts", bufs=1))
        p_seq = ctx.enter_context(tc.tile_pool(name="p_seq", bufs=tn["seq"]))
        p_x = ctx.enter_context(tc.tile_pool(name="p_x", bufs=tn["x"]))
        p_big = ctx.enter_context(tc.tile_pool(name="p_big", bufs=tn["big"]))
        p_med = ctx.enter_context(tc.tile_pool(name="p_med", bufs=tn["med"]))
        p_sm = ctx.enter_context(tc.tile_pool(name="p_sm", bufs=tn["sm"]))
        p_keep = ctx.enter_context(tc.tile_pool(name="p_keep", bufs=1))
        # PSUM: 8 banks total
        ps_xt = ctx.enter_context(tc.tile_pool(name="ps_xt", bufs=2, space="PSUM"))
        ps_gi = ctx.enter_context(tc.tile_pool(
            name="ps_gi", bufs=1 if tn["gru1024"] else 2, space="PSUM"))
        ps_sc = ctx.enter_context(tc.tile_pool(name="ps_sc", bufs=1, space="PSUM"))
        ps_v = ctx.enter_context(tc.tile_pool(name="ps_v", bufs=1, space="PSUM"))
        ps_sm = ctx.enter_context(tc.tile_pool(name="ps_sm", bufs=1, space="PSUM"))

        # ---- load constants ----
        eye = consts.tile([128, 128], F32)
        nc.sync.dma_start(out=eye, in_=t_eye[:, :])
        eyebf = consts.tile([128, 128], BF16)
        nc.sync.dma_start(out=eyebf, in_=t_eyebf[:, :])
        w_sb = {}
        for n in wnames:
            w_sb[n] = consts.tile([128, 128], BF16, name=f"w_{n}")
            nc.sync.dma_start(out=w_sb[n], in_=t_w[n][:, :])
        v_sb = {}
        for n in vnames:
            v_sb[n] = consts.tile([128, 1], F32, name=f"v_{n}")
            nc.sync.dma_start(out=v_sb[n], in_=t_v[n][:, :])
        ones2 = consts.tile([128, 2], BF16)
        nc.sync.dma_start(out=ones2, in_=t_ones2[:, :])
        ones_row = consts.tile([1, 128], BF16)
        nc.sync.dma_start(out=ones_row, in_=t_onesr[:, :])

        def transpose(out_ap, in_ap, ident, base=0, k=128):
            tp = (base, 0) if base else None
            nc.tensor.transpose(out_ap, in_ap, ident[base:base + k, base:base + k],
                                tile_position=tp)

        # ---- precompute: srcT (bf16) and q for all supertiles ----
        sb_src_rm = p_keep.tile([128, 8, 128], BF16, tag="src_rm")
        nc.gpsimd.dma_start(out=sb_src_rm, in_=src_v)
        sb_srcT = p_keep.tile([128, SRC_ROWS], BF16, tag="srcT")
        for r in range(2):
            pt_st = ps_xt.tile([128, 512], BF16, tag="xt", name=f"pt_src{r}")
            for t4 in range(4):
                transpose(pt_st[:, t4 * 128:(t4 + 1) * 128],
                          sb_src_rm[:, r * 4 + t4, :], eyebf)
            nc.vector.tensor_copy(out=sb_srcT[:, r * 512:(r + 1) * 512], in_=pt_st)
        sb_q = p_keep.tile([128, SRC_ROWS], BF16, tag="q_all")
        pt_q = ps_sc.tile([128, 1024], F32, tag="sc", name="pt_q")
        nc.tensor.matmul(pt_q[:, 0:512], w_sb["wqT"], sb_srcT[:, 0:512],
                         start=True, stop=True)
        nc.tensor.matmul(pt_q[:, 512:1024], w_sb["wqT"], sb_srcT[:, 512:1024],
                         start=True, stop=True)
        nc.vector.tensor_copy(out=sb_q, in_=pt_q)

        sb_oaT_all = p_keep.tile([128, N_ST * 128], BF16, tag="oaT_all")

        def gru_phase(st):
            sb_seqT = p_seq.tile([128, ST_ROWS], BF16, tag="seqT")
            x_bf = p_x.tile([128, 32, 128], BF16, tag="x_bf")
            xv = x_bf[:, :, :].rearrange("p (ac j) d -> p ac j d", ac=4)
            nc.gpsimd.dma_start(out=xv, in_=ngh_v[st])
            for gc in range(N_GC):
                pt_xt = ps_xt.tile([128, 512], BF16, tag="xt")
                for t in range(4):
                    transpose(pt_xt[:, t * 128:(t + 1) * 128],
                              x_bf[:, gc * 4 + t, :], eyebf)
                sb_xT = p_med.tile([128, 512], BF16, tag="xT")
                if (st * N_GC + gc) % tn["xtd_mod"] == 0:
                    nc.scalar.copy(out=sb_xT, in_=pt_xt)
                else:
                    nc.vector.tensor_copy(out=sb_xT, in_=pt_xt)
                if tn["gru1024"]:
                    # 0.5 for the z-gate is folded into wihzT host-side, so
                    # one tanh covers both gates in a 2-bank PSUM tile
                    pt_g = ps_gi.tile([128, 1024], F32, tag="gi", name="pt_g")
                    nc.tensor.matmul(pt_g[:, 0:512], w_sb["wihzT"], sb_xT,
                                     start=True, stop=True)
                    nc.tensor.matmul(pt_g[:, 512:1024], w_sb["wihnT"], sb_xT,
                                     start=True, stop=True)
                    sb_tab = p_med.tile([128, 1024], BF16, tag="tab")
                    nc.scalar.activation(out=sb_tab, in_=pt_g, func=AF.Tanh)
                    sb_ta = sb_tab[:, 0:512]
                    sb_tb = sb_tab[:, 512:1024]
                else:
                    pt_gz = ps_gi.tile([128, 512], F32, tag="gi", name="pt_gz")
                    nc.tensor.matmul(pt_gz, w_sb["wihzT"], sb_xT, start=True, stop=True)
                    pt_gn = ps_gi.tile([128, 512], F32, tag="gi", name="pt_gn")
                    nc.tensor.matmul(pt_gn, w_sb["wihnT"], sb_xT, start=True, stop=True)
                    sb_ta = p_med.tile([128, 512], BF16, tag="ta")
                    nc.scalar.activation(out=sb_ta, in_=pt_gz, func=AF.Tanh, scale=0.5)
                    sb_tb = p_med.tile([128, 512], BF16, tag="tb")
                    nc.scalar.activation(out=sb_tb, in_=pt_gn, func=AF.Tanh)
                # seq' = (ta - 1) * tb  (= -2 * sigmoid(-gz) * tanh(gn))
                if tn["gp_stt"] and gc % 2 == 1:
                    # offload to GpSimd as mul+sub (STT unsupported on Pool)
                    sb_p = p_med.tile([128, 512], BF16, tag="gp_p")
                    nc.gpsimd.tensor_mul(sb_p, sb_ta, sb_tb)
                    nc.gpsimd.tensor_sub(sb_seqT[:, gc * 512:(gc + 1) * 512],
                                         sb_p, sb_tb)
                else:
                    nc.vector.scalar_tensor_tensor(
                        out=sb_seqT[:, gc * 512:(gc + 1) * 512],
                        in0=sb_ta, scalar=1.0, in1=sb_tb,
                        op0=ALU.subtract, op1=ALU.mult)
            # q' = wks^T @ qemb (head-embedded q columns for this supertile)
            sb_qemb = p_sm.tile([128, 128], BF16, tag="qemb")
            nc.gpsimd.memset(sb_qemb, 0.0)
            qe_v = sb_qemb[:, :].rearrange("p (cg h) -> p h cg", h=2)
            nc.vector.tensor_copy(out=qe_v[0:64, 0, :],
                                  in_=sb_q[0:64, st * 64:(st + 1) * 64])
            nc.vector.tensor_copy(out=qe_v[64:128, 1, :],
                                  in_=sb_q[64:128, st * 64:(st + 1) * 64])
            pt_qp = ps_v.tile([128, 128], F32, tag="v", name="pt_qp")
            nc.tensor.matmul(pt_qp, w_sb["wks"], sb_qemb, start=True, stop=True)
            sb_qp = p_sm.tile([128, 128], BF16, tag="qp", bufs=3)
            nc.vector.tensor_copy(out=sb_qp, in_=pt_qp)
            return sb_seqT, sb_qp

        def attn_phase(st, sb_seqT, sb_qp):
            sb_mask = p_big.tile([128, 1024], BF16, tag="mask")
            nc.sync.dma_start(out=sb_mask, in_=t_mask[st])

            # column-tiled scores: partitions 32ac..32ac+32 <- q'_ac x seqT_ac
            # the additive mask is pre-accumulated into PSUM via an identity
            # matmul, so exp can read PSUM directly (no DVE mask add)
            pt_sc = ps_sc.tile([128, 1024], F32, tag="sc")
            pe_mask = tn["pe_mask"]
            if pe_mask:
                for half in range(2):
                    nc.tensor.matmul(
                        pt_sc[:, half * 512:(half + 1) * 512],
                        eyebf, sb_mask[:, half * 512:(half + 1) * 512],
                        start=True, stop=False, skip_group_check=True)
            for ac in range(N_AC):
                for half in range(2):
                    nc.tensor.matmul(
                        pt_sc[32 * ac:32 * ac + 32, half * 512:(half + 1) * 512],
                        sb_qp[:, 32 * ac:32 * ac + 32],
                        sb_seqT[:, ac * 1024 + half * 512:
                                ac * 1024 + (half + 1) * 512],
                        start=not pe_mask, stop=True,
                        tile_position=(0, 32 * ac), skip_group_check=True)

            sb_attn = p_big.tile([128, 1024], BF16, tag="attn")
            sb_sums = p_sm.tile([128, 1], F32, tag="sums")
            if pe_mask:
                nc.scalar.activation(out=sb_attn, in_=pt_sc, func=AF.Exp,
                                     accum_out=sb_sums)
            else:
                sb_scm = p_big.tile([128, 1024], BF16, tag="scm")
                nc.vector.tensor_add(sb_scm, pt_sc, sb_mask)
                nc.scalar.activation(out=sb_attn, in_=sb_scm, func=AF.Exp,
                                     accum_out=sb_sums)
            sb_sum2 = p_sm.tile([128, 1], F32, tag="sums", name="sb_sum2")
            nc.vector.tensor_scalar_add(sb_sum2, sb_sums, 1e-30)
            sb_rec = p_sm.tile([128, 1], F32, tag="rec")
            nc.vector.reciprocal(sb_rec, sb_sum2)
            sb_attn_n = p_big.tile([128, 1024], BF16, tag="attn_n")
            nc.vector.tensor_scalar_mul(sb_attn_n, sb_attn, sb_rec)

            # transpose attn in full-width [128,128] tiles (all 4 ac blocks
            # share the same within-block column space)
            at_chunks = []
            for jg in range(8):
                pt_at = ps_v.tile([128, 128], BF16, tag="v", name="pt_atf")
                transpose(pt_at, sb_attn_n[:, jg * 128:(jg + 1) * 128], eyebf)
                sb_atf = p_sm.tile([128, 128], BF16, tag="at", bufs=9)
                nc.vector.tensor_copy(out=sb_atf, in_=pt_at)
                at_chunks.append(sb_atf)
            pt_oaT = ps_sm.tile([128, 128], F32, tag="sm", name="pt_oaT")
            for ac in range(N_AC):
                base = ac * 1024
                # row-major seq chunks via PE transpose; W_v is folded into
                # the fc weights host-side, so oaT = sum_j seq_j^T-rows @ at_j
                sb_vrm = p_sm.tile([128, 8, 128], BF16, tag="vrm", bufs=2)
                for half in range(2):
                    pt_v = ps_v.tile([128, 512], BF16, tag="v")
                    for j in range(4):
                        sl = sb_seqT[:, base + half * 512 + j * 128:
                                     base + half * 512 + (j + 1) * 128]
                        transpose(pt_v[:, j * 128:(j + 1) * 128], sl, eyebf)
                    nc.vector.tensor_copy(out=sb_vrm[:, half * 4:(half + 1) * 4, :],
                                          in_=pt_v)
                # oaT[:, 32ac:32ac+32] = sum_j seq_j^T @ at_j   ([feat, slot])
                for j in range(8):
                    nc.tensor.matmul(pt_oaT[:, 32 * ac:32 * ac + 32],
                                     sb_vrm[:, j, :],
                                     at_chunks[j][:, 32 * ac:32 * ac + 32],
                                     start=(j == 0), stop=(j == 7))
            nc.vector.tensor_copy(out=sb_oaT_all[:, st * 128:(st + 1) * 128],
                                  in_=pt_oaT)

        # ---- batched tail: fc + residual + LN + merge MLP, 2 segments ----
        oa_v = sb_oaT_all[:, :].rearrange("p (r h) -> p h r", h=2)

        def tail_seg(seg):
            rsl = slice(seg * 512, (seg + 1) * 512)
            pt_fc = ps_sc.tile([128, 512], F32, tag="sc", name="pt_fc")
            nc.tensor.matmul(pt_fc, w_sb["fcw0T"], oa_v[:, 0, rsl],
                             start=True, stop=False)
            nc.tensor.matmul(pt_fc, w_sb["fcw1T"], oa_v[:, 1, rsl],
                             start=False, stop=True)
            sb_x1 = p_med.tile([128, 512], BF16, tag="x1")
            nc.vector.tensor_scalar_add(sb_x1, pt_fc, v_sb["fc_b"])
            sb_x2 = p_med.tile([128, 512], BF16, tag="x2", bufs=2)
            nc.vector.tensor_add(sb_x2, sb_x1, sb_srcT[:, rsl])
            sb_sq = p_med.tile([128, 512], BF16, tag="sq")
            nc.scalar.activation(out=sb_sq, in_=sb_x2, func=AF.Square)
            pt_ln = ps_sc.tile([2, 1024], F32, tag="sc", name="pt_ln")
            nc.tensor.matmul(pt_ln[0:2, 0:512], ones2, sb_x2,
                             start=True, stop=True)
            nc.tensor.matmul(pt_ln[0:2, 512:1024], ones2, sb_sq,
                             start=True, stop=True)
            sb_mu = p_sm.tile([1, 512], F32, tag="mu")
            nc.vector.tensor_scalar_mul(sb_mu, pt_ln[0:1, 0:512], 1.0 / 128.0)
            sb_ex2 = p_sm.tile([1, 512], F32, tag="ex2")
            nc.vector.tensor_scalar(sb_ex2, pt_ln[0:1, 512:1024],
                                    1.0 / 128.0, LN_EPS,
                                    op0=ALU.mult, op1=ALU.add)
            sb_musq = p_sm.tile([1, 512], F32, tag="musq")
            nc.vector.tensor_mul(sb_musq, sb_mu, sb_mu)
            sb_ve = p_sm.tile([1, 512], F32, tag="ve")
            nc.vector.tensor_sub(sb_ve, sb_ex2, sb_musq)
            sb_stats = p_sm.tile([1, 1024], BF16, tag="stats")
            nc.vector.tensor_copy(out=sb_stats[0:1, 0:512], in_=sb_mu)
            sb_rve = p_sm.tile([1, 512], F32, tag="rve")
            nc.vector.reciprocal(sb_rve, sb_ve)
            nc.scalar.activation(out=sb_stats[0:1, 512:1024], in_=sb_rve,
                                 func=AF.Sqrt)
            pt_bc = ps_sc.tile([128, 1024], F32, tag="sc", name="pt_bc")
            nc.tensor.matmul(pt_bc[:, 0:512], ones_row, sb_stats[0:1, 0:512],
                             start=True, stop=True)
            nc.tensor.matmul(pt_bc[:, 512:1024], ones_row,
                             sb_stats[0:1, 512:1024], start=True, stop=True)
            sb_xc = p_med.tile([128, 512], BF16, tag="xc")
            nc.vector.tensor_sub(sb_xc, sb_x2, pt_bc[:, 0:512])
            sb_xn0 = p_med.tile([128, 512], BF16, tag="xn0")
            nc.vector.tensor_mul(sb_xn0, sb_xc, pt_bc[:, 512:1024])
            sb_xn = p_med.tile([128, 512], BF16, tag="xn")
            nc.vector.tensor_scalar(sb_xn, sb_xn0, v_sb["ln_g"], v_sb["ln_b"],
                                    op0=ALU.mult, op1=ALU.add)
            pt_h1 = ps_gi.tile([128, 512], F32, tag="gi", name="pt_h1")
            nc.tensor.matmul(pt_h1, w_sb["m1aT"], sb_xn, start=True, stop=False)
            nc.tensor.matmul(pt_h1, w_sb["m1bT"], sb_srcT[:, rsl],
                             start=False, stop=True)
            sb_h1 = p_med.tile([128, 512], BF16, tag="h1")
            nc.scalar.activation(out=sb_h1, in_=pt_h1, func=AF.Relu,
                                 bias=v_sb["m1b"])
            pt_z = ps_gi.tile([128, 512], F32, tag="gi", name="pt_z")
            nc.tensor.matmul(pt_z, w_sb["m2T"], sb_h1, start=True, stop=True)
            sb_zb = p_med.tile([128, 512], F32, tag="zb")
            nc.vector.tensor_scalar_add(sb_zb, pt_z, v_sb["m2b"])
            pt_zr = ps_xt.tile([128, 512], F32, tag="xt", name="pt_zr")
            for t in range(4):
                transpose(pt_zr[:, t * 128:(t + 1) * 128],
                          sb_zb[:, t * 128:(t + 1) * 128], eye)
            sb_zout = p_med.tile([128, 4, 128], F32, tag="zout")
            nc.vector.tensor_copy(out=sb_zout, in_=pt_zr)
            nc.sync.dma_start(out=out_v[:, seg * 4:(seg + 1) * 4, :], in_=sb_zout)

        for st in range(N_ST):
            carry = gru_phase(st)
            attn_phase(st, *carry)
            if st == 7:
                tail_seg(0)
            elif st == 15:
                tail_seg(1)

    nc.finalize()
    return nc


# ----------------------------------------------------------------------------
# v1 general-path program (f32r, handles nonzero hidden / gru biases)
# ----------------------------------------------------------------------------

def build_program_v1(general: bool, use_f32r: bool = True, xt_copy: str = "act"):
    tn = {"seq": 5, "stl": 5, "chunk": 3, "att": 2, "sm": 3,
          "xt": 1, "gi": 2, "v": 1, "sc": 1, "psm": 2}
    nc = bacc.Bacc("TRN2")
    MMDT = F32R if use_f32r else F32

    t_ngh = nc.dram_tensor("ngh", [ROWS, D], F32, kind="ExternalInput")
    t_src = nc.dram_tensor("srcf", [B_CORE * N_SRC, D], F32, kind="ExternalInput")
    t_mask = nc.dram_tensor("maskfull", [N_ST, 128, 1024], BF16, kind="ExternalInput")
    t_eye = nc.dram_tensor("eye", [128, 128], F32, kind="ExternalInput")
    wnames = ["wqT", "wks", "wihzT", "wihnT", "wvsT",
              "fcw0T", "fcw1T", "m1aT", "m1bT", "m2T"]
    if general:
        wnames += ["wihrT", "whhrT", "whhzT", "whhnT"]
    t_w = {n: nc.dram_tensor(n, [128, 128], MMDT, kind="ExternalInput") for n in wnames}
    t_onesc = nc.dram_tensor("ones2", [128, 2], MMDT, kind="ExternalInput")
    t_onesr = nc.dram_tensor("onesrow", [1, 128], MMDT, kind="ExternalInput")
    t_zeros = nc.dram_tensor("zeros128", [128, 128], MMDT, kind="ExternalInput")
    vnames = ["fc_b", "ln_g", "ln_b", "m1b", "m2b"]
    if general:
        vnames += ["b_r", "b_z", "b_in", "b_hn"]
    t_v = {n: nc.dram_tensor(n, [128, 1], F32, kind="ExternalInput") for n in vnames}
    if general:
        t_hid = nc.dram_tensor("hid", [ROWS, D], F32, kind="ExternalInput")
    t_out = nc.dram_tensor("z", [B_CORE * N_SRC, D], F32, kind="ExternalOutput")

    ngh_v = t_ngh[:, :].rearrange("(st gc t p) d -> st gc p t d", st=N_ST, gc=N_GC, t=4, p=128)
    if general:
        hid_v = t_hid[:, :].rearrange("(st gc t p) d -> st gc p t d", st=N_ST, gc=N_GC, t=4, p=128)

    with tile.TileContext(nc) as tc, ExitStack() as ctx:
        consts = ctx.enter_context(tc.tile_pool(name="consts", bufs=1))
        p_seq = ctx.enter_context(tc.tile_pool(name="p_seq", bufs=tn["seq"]))
        p_stl = ctx.enter_context(tc.tile_pool(name="p_stl", bufs=tn["stl"]))
        p_chunk = ctx.enter_context(tc.tile_pool(name="p_chunk", bufs=tn["chunk"]))
        p_att = ctx.enter_context(tc.tile_pool(name="p_att", bufs=tn["att"]))
        p_sm = ctx.enter_context(tc.tile_pool(name="p_sm", bufs=tn["sm"]))
        ps_xt = ctx.enter_context(tc.tile_pool(name="ps_xt", bufs=tn["xt"], space="PSUM"))
        ps_gi = ctx.enter_context(tc.tile_pool(name="ps_gi", bufs=tn["gi"], space="PSUM"))
        ps_v = ctx.enter_context(tc.tile_pool(name="ps_v", bufs=tn["v"], space="PSUM"))
        ps_sc = ctx.enter_context(tc.tile_pool(name="ps_sc", bufs=tn["sc"], space="PSUM"))
        ps_at = ctx.enter_context(tc.tile_pool(name="ps_at", bufs=1, space="PSUM"))
        ps_sm = ctx.enter_context(tc.tile_pool(name="ps_sm", bufs=tn["psm"], space="PSUM"))

        eye = consts.tile([128, 128], F32)
        nc.sync.dma_start(out=eye, in_=t_eye[:, :])
        w_sb = {}
        for n in wnames:
            w_sb[n] = consts.tile([128, 128], MMDT, name=f"w_{n}")
            nc.sync.dma_start(out=w_sb[n], in_=t_w[n][:, :])
        v_sb = {}
        for n in vnames:
            v_sb[n] = consts.tile([128, 1], F32, name=f"v_{n}")
            nc.sync.dma_start(out=v_sb[n], in_=t_v[n][:, :])
        ones2 = consts.tile([128, 2], MMDT)
        nc.sync.dma_start(out=ones2, in_=t_onesc[:, :])
        zconst = consts.tile([128, 128], MMDT)
        nc.sync.dma_start(out=zconst, in_=t_zeros[:, :])
        ones_row = consts.tile([1, 128], MMDT)
        nc.sync.dma_start(out=ones_row, in_=t_onesr[:, :])

        def transpose(out_ap, in_ap, base=0, k=128):
            tp = (base, 0) if base else None
            nc.tensor.transpose(out_ap, in_ap, eye[base:base + k, base:base + k],
                                tile_position=tp)

        def gru_phase(st):
            sb_src = p_sm.tile([64, 128], F32, tag="src")
            nc.sync.dma_start(out=sb_src, in_=t_src[st * 64:(st + 1) * 64, :])
            pt_srcT = ps_sm.tile([128, 128], F32, tag="psmall", name="pt_srcT")
            transpose(pt_srcT[:, 0:64], sb_src, k=64)
            sb_srcT = p_stl.tile([128, 64], MMDT, tag="srcT")
            nc.scalar.copy(out=sb_srcT, in_=pt_srcT[:, 0:64])

            pt_q = ps_sm.tile([128, 128], F32, tag="psmall", name="pt_q")
            nc.tensor.matmul(pt_q[:, 0:64], w_sb["wqT"],
                             sb_srcT, start=True, stop=True)
            sb_qT = p_sm.tile([128, 64], F32, tag="qT")
            nc.vector.tensor_copy(out=sb_qT, in_=pt_q[:, 0:64])
            sb_qemb = p_sm.tile([128, 128], MMDT, tag="qemb")
            nc.sync.dma_start(out=sb_qemb, in_=t_zeros[:, :])
            qe_v = sb_qemb[:, :].rearrange("p (cg h) -> p h cg", h=2)
            nc.vector.tensor_copy(out=qe_v[0:64, 0, :], in_=sb_qT[0:64, :])
            nc.vector.tensor_copy(out=qe_v[64:128, 1, :], in_=sb_qT[64:128, :])
            pt_qp = ps_sm.tile([128, 128], F32, tag="psmall", name="pt_qp")
            nc.tensor.matmul(pt_qp, w_sb["wks"],
                             sb_qemb, start=True, stop=True)
            sb_qp = []
            for c in range(N_AC):
                qz = p_stl.tile([128, 128], MMDT, tag="qpz", bufs=8, name=f"qz{c}")
                nc.sync.dma_start(out=qz, in_=zconst)
                nc.vector.tensor_copy(out=qz[:, 32 * c:32 * c + 32],
                                      in_=pt_qp[:, 32 * c:32 * c + 32])
                sb_qp.append(qz)

            sb_seqT = p_seq.tile([128, ST_ROWS], MMDT, tag="seqT")
            for gc in range(N_GC):
                x_rm = p_chunk.tile([128, 4, 128], F32, tag="x_rm")
                nc.sync.dma_start(out=x_rm, in_=ngh_v[st, gc])
                pt_xt = ps_xt.tile([128, 512], F32, tag="xt")
                for t in range(4):
                    transpose(pt_xt[:, t * 128:(t + 1) * 128], x_rm[:, t, :])
                sb_xT = p_chunk.tile([128, 512], MMDT, tag="xT")
                nc.scalar.copy(out=sb_xT, in_=pt_xt)
                h_rm = p_chunk.tile([128, 4, 128], F32, tag="h_rm")
                nc.sync.dma_start(out=h_rm, in_=hid_v[st, gc])
                pt_ht = ps_xt.tile([128, 512], F32, tag="xt", name="pt_ht")
                for t in range(4):
                    transpose(pt_ht[:, t * 128:(t + 1) * 128], h_rm[:, t, :])
                sb_hT = p_chunk.tile([128, 512], MMDT, tag="hT")
                nc.scalar.copy(out=sb_hT, in_=pt_ht)

                seq_sl = sb_seqT[:, gc * 512:(gc + 1) * 512]
                pt_gr = ps_gi.tile([128, 512], F32, tag="gi", name="pt_gr")
                nc.tensor.matmul(pt_gr, w_sb["wihrT"],
                                 sb_xT, start=True, stop=False)
                nc.tensor.matmul(pt_gr, w_sb["whhrT"],
                                 sb_hT, start=False, stop=True)
                pt_gz = ps_gi.tile([128, 512], F32, tag="gi", name="pt_gz")
                nc.tensor.matmul(pt_gz, w_sb["wihzT"],
                                 sb_xT, start=True, stop=False)
                nc.tensor.matmul(pt_gz, w_sb["whhzT"],
                                 sb_hT, start=False, stop=True)
                pt_gni = ps_gi.tile([128, 512], F32, tag="gi", name="pt_gni")
                nc.tensor.matmul(pt_gni, w_sb["wihnT"],
                                 sb_xT, start=True, stop=True)
                pt_gnh = ps_gi.tile([128, 512], F32, tag="gi", name="pt_gnh")
                nc.tensor.matmul(pt_gnh, w_sb["whhnT"],
                                 sb_hT, start=True, stop=True)
                sb_r = p_chunk.tile([128, 512], F32, tag="zc", name="sb_r")
                nc.scalar.activation(out=sb_r, in_=pt_gr, func=AF.Sigmoid,
                                     bias=v_sb["b_r"])
                sb_z = p_chunk.tile([128, 512], F32, tag="zc", name="sb_z")
                nc.scalar.activation(out=sb_z, in_=pt_gz, func=AF.Sigmoid,
                                     bias=v_sb["b_z"])
                sb_hnb = p_chunk.tile([128, 512], F32, tag="nn", name="sb_hnb")
                nc.vector.tensor_scalar_add(sb_hnb, pt_gnh, v_sb["b_hn"])
                sb_rn = p_chunk.tile([128, 512], F32, tag="nn", name="sb_rn")
                nc.vector.tensor_mul(sb_rn, sb_r, sb_hnb)
                sb_np = p_chunk.tile([128, 512], F32, tag="nn", name="sb_np")
                nc.vector.tensor_add(sb_np, pt_gni, sb_rn)
                sb_nn = p_chunk.tile([128, 512], F32, tag="nn", name="sb_nn")
                nc.scalar.activation(out=sb_nn, in_=sb_np, func=AF.Tanh,
                                     bias=v_sb["b_in"])
                sb_hmn = p_chunk.tile([128, 512], F32, tag="nn", name="sb_hmn")
                nc.vector.tensor_sub(sb_hmn, sb_hT, sb_nn)
                sb_zh = p_chunk.tile([128, 512], F32, tag="nn", name="sb_zh")
                nc.vector.tensor_mul(sb_zh, sb_z, sb_hmn)
                nc.vector.tensor_add(seq_sl, sb_nn, sb_zh)
            return sb_srcT, sb_qp, sb_seqT

        def attn_phase(st, sb_srcT, sb_qp, sb_seqT):
            sb_mask = p_att.tile([128, 1024], BF16, tag="mask")
            nc.sync.dma_start(out=sb_mask, in_=t_mask[st])

            pt_sc = ps_sc.tile([128, 1024], F32, tag="sc")
            for ac in range(N_AC):
                base = ac * 1024
                for half in range(2):
                    nc.tensor.matmul(
                        pt_sc[:, half * 512:(half + 1) * 512],
                        sb_qp[ac],
                        sb_seqT[:, base + half * 512:base + (half + 1) * 512],
                        start=(ac == 0), stop=(ac == N_AC - 1))

            sb_scm = p_att.tile([128, 1024], F32, tag="scm")
            nc.vector.tensor_add(sb_scm, pt_sc, sb_mask)
            sb_attn = p_att.tile([128, 1024], F32, tag="attn")
            sb_sums = p_sm.tile([128, 1], F32, tag="sums")
            nc.scalar.activation(out=sb_attn, in_=sb_scm, func=AF.Exp,
                                 accum_out=sb_sums)
            sb_rec = p_stl.tile([128, 1], F32, tag="rec")
            sb_sum2 = p_sm.tile([128, 1], F32, tag="sums", name="sb_sum2")
            nc.vector.tensor_scalar_add(sb_sum2, sb_sums, 1e-30)
            nc.vector.reciprocal(sb_rec, sb_sum2)
            sb_attn_n = p_att.tile([128, 1024], F32, tag="attn_n")
            nc.vector.tensor_scalar_mul(sb_attn_n, sb_attn, sb_rec)

            pt_oaT = ps_sm.tile([128, 128], F32, tag="psmall", name="pt_oaT")
            for ac in range(N_AC):
                base = ac * 1024
                sb_vrm = p_att.tile([128, 8, 128], MMDT, tag="vrm", bufs=2)
                for half in range(2):
                    pt_v = ps_v.tile([128, 512], F32, tag="v")
                    for j in range(4):
                        sl = sb_seqT[:, base + half * 512 + j * 128:
                                     base + half * 512 + (j + 1) * 128]
                        nc.tensor.matmul(pt_v[:, j * 128:(j + 1) * 128],
                                         sl, w_sb["wvsT"],
                                         start=True, stop=True)
                    nc.vector.tensor_copy(out=sb_vrm[:, half * 4:(half + 1) * 4, :],
                                          in_=pt_v)
                pt_at = ps_v.tile([128, 256], F32, tag="v", name="pt_at")
                for j in range(8):
                    transpose(pt_at[:, j * 32:(j + 1) * 32],
                              sb_attn_n[32 * ac:32 * ac + 32, j * 128:(j + 1) * 128],
                              base=32 * ac, k=32)
                sb_at = p_sm.tile([128, 256], MMDT, tag="at")
                nc.vector.tensor_copy(out=sb_at, in_=pt_at)
                pt_oa = ps_sm.tile([32, 128], F32, tag="psmall", name="pt_oa")
                for j in range(8):
                    nc.tensor.matmul(pt_oa,
                                     sb_at[:, j * 32:(j + 1) * 32],
                                     sb_vrm[:, j, :],
                                     start=(j == 0), stop=(j == 7))
                sb_oa = p_sm.tile([32, 128], F32, tag="oa")
                nc.vector.tensor_copy(out=sb_oa, in_=pt_oa)
                transpose(pt_oaT[:, 32 * ac:32 * ac + 32], sb_oa, k=32)
            sb_oaT = p_sm.tile([128, 128], MMDT, tag="oaT")
            nc.vector.tensor_copy(out=sb_oaT, in_=pt_oaT)

            oaT_v = sb_oaT[:, :].rearrange("p (cg h) -> p h cg", h=2)
            pt_fc = ps_sm.tile([128, 128], F32, tag="psmall", name="pt_fc")
            nc.tensor.matmul(pt_fc[:, 0:64], w_sb["fcw0T"],
                             oaT_v[:, 0, :], start=True, stop=False)
            nc.tensor.matmul(pt_fc[:, 0:64], w_sb["fcw1T"],
                             oaT_v[:, 1, :], start=False, stop=True)

            sb_x1 = p_sm.tile([128, 64], F32, tag="x1")
            nc.vector.tensor_scalar_add(sb_x1, pt_fc[:, 0:64], v_sb["fc_b"])
            sb_x2 = p_sm.tile([128, 64], MMDT, tag="x2")
            nc.vector.tensor_add(sb_x2, sb_x1, sb_srcT)
            sb_sq = p_sm.tile([128, 64], MMDT, tag="sq")
            nc.scalar.activation(out=sb_sq, in_=sb_x2[:, :].bitcast(F32), func=AF.Square)
            pt_ln = ps_sm.tile([128, 128], F32, tag="psmall", name="pt_ln")
            nc.tensor.matmul(pt_ln[0:2, 0:64], ones2,
                             sb_x2, start=True, stop=True)
            nc.tensor.matmul(pt_ln[0:2, 64:128], ones2,
                             sb_sq, start=True, stop=True)
            sb_stats = p_sm.tile([1, 128], MMDT, tag="ln_stats")
            sb_mu = sb_stats[0:1, 0:64]
            nc.vector.tensor_scalar_mul(sb_mu, pt_ln[0:1, 0:64], 1.0 / 128.0)
            sb_ve = p_sm.tile([1, 64], F32, tag="ln_ve")
            sb_ex2 = p_sm.tile([1, 64], F32, tag="ln_ex2")
            nc.vector.tensor_scalar(sb_ex2, pt_ln[0:1, 64:128], 1.0 / 128.0, LN_EPS,
                                    op0=ALU.mult, op1=ALU.add)
            sb_musq = p_sm.tile([1, 64], F32, tag="ln_musq")
            nc.vector.tensor_mul(sb_musq, sb_mu, sb_mu)
            nc.vector.tensor_sub(sb_ve, sb_ex2, sb_musq)
            sb_y = p_sm.tile([1, 64], F32, tag="ln_y")
            sb_yi = p_sm.tile([1, 64], I32, tag="ln_yi")
            nc.vector.tensor_scalar(sb_yi, sb_ve[:, :].bitcast(I32), 1, None,
                                    op0=ALU.arith_shift_right)
            nc.vector.tensor_scalar(sb_y[:, :].bitcast(I32), sb_yi, -1, 0x5F3759DF,
                                    op0=ALU.mult, op1=ALU.add)
            for it in range(3):
                sb_t = p_sm.tile([1, 64], F32, tag="ln_t")
                nc.vector.tensor_mul(sb_t, sb_y, sb_y)
                sb_t2 = p_sm.tile([1, 64], F32, tag="ln_t2")
                nc.vector.tensor_mul(sb_t2, sb_t, sb_ve)
                sb_t3 = p_sm.tile([1, 64], F32, tag="ln_t3")
                nc.vector.tensor_scalar(sb_t3, sb_t2, -0.5, 1.5, op0=ALU.mult, op1=ALU.add)
                if it < 2:
                    sb_y2 = p_sm.tile([1, 64], F32, tag="ln_y2")
                else:
                    sb_y2 = sb_stats[0:1, 64:128]
                nc.vector.tensor_mul(sb_y2, sb_y, sb_t3)
                sb_y = sb_y2
            pt_bc = ps_sm.tile([128, 128], F32, tag="psmall", name="pt_bc")
            nc.tensor.matmul(pt_bc, ones_row, sb_stats,
                             start=True, stop=True)
            sb_xc = p_sm.tile([128, 64], F32, tag="xc")
            nc.vector.tensor_sub(sb_xc, sb_x2, pt_bc[:, 0:64])
            sb_xn0 = p_sm.tile([128, 64], F32, tag="xn0")
            nc.vector.tensor_mul(sb_xn0, sb_xc, pt_bc[:, 64:128])
            sb_xn = p_sm.tile([128, 64], MMDT, tag="xn")
            nc.vector.tensor_scalar(sb_xn, sb_xn0, v_sb["ln_g"], v_sb["ln_b"],
                                    op0=ALU.mult, op1=ALU.add)

            pt_h1 = ps_sm.tile([128, 128], F32, tag="psmall", name="pt_h1")
            nc.tensor.matmul(pt_h1[:, 0:64], w_sb["m1aT"],
                             sb_xn, start=True, stop=False)
            nc.tensor.matmul(pt_h1[:, 0:64], w_sb["m1bT"],
                             sb_srcT, start=False, stop=True)
            sb_h1 = p_sm.tile([128, 64], MMDT, tag="h1")
            nc.scalar.activation(out=sb_h1, in_=pt_h1[:, 0:64], func=AF.Relu,
                                 bias=v_sb["m1b"])
            pt_z = ps_sm.tile([128, 128], F32, tag="psmall", name="pt_z")
            nc.tensor.matmul(pt_z[:, 0:64], w_sb["m2T"],
                             sb_h1, start=True, stop=True)
            sb_zb = p_sm.tile([128, 64], F32, tag="zb")
            nc.vector.tensor_scalar_add(sb_zb, pt_z[:, 0:64], v_sb["m2b"])
            pt_zr = ps_sm.tile([128, 128], F32, tag="psmall", name="pt_zr")
            transpose(pt_zr[0:64, :], sb_zb)
            sb_zout = p_sm.tile([64, 128], F32, tag="zout")
            nc.scalar.copy(out=sb_zout, in_=pt_zr[0:64, :])
            nc.sync.dma_start(out=t_out[st * 64:(st + 1) * 64, :], in_=sb_zout)

        for grp in range(N_ST // 4):
            sts = range(grp * 4, (grp + 1) * 4)
            carry = [gru_phase(st) for st in sts]
            for st, c in zip(sts, carry):
                attn_phase(st, *c)

    nc.finalize()
    return nc


# ----------------------------------------------------------------------------
# Host side
# ----------------------------------------------------------------------------

def _prep_inputs(inputs, general):
    """Build per-core input maps (numpy) from full-size inputs."""
    f32 = np.float32
    bf16 = ml_dtypes.bfloat16
    src = np.ascontiguousarray(np.asarray(inputs["src"], f32))
    ngh = np.ascontiguousarray(np.asarray(inputs["ngh_feat"], f32))
    mask = np.asarray(inputs["mask"]).astype(bool)
    w_qs = np.asarray(inputs["w_qs"], f32)
    w_ks = np.asarray(inputs["w_ks"], f32)
    w_vs = np.asarray(inputs["w_vs"], f32)
    fc_w = np.asarray(inputs["fc_w"], f32)
    w_ih = np.asarray(inputs["gru_w_ih"], f32)
    m_fc1 = np.asarray(inputs["m_fc1_w"], f32)
    m_fc2 = np.asarray(inputs["m_fc2_w"], f32)

    wdt = f32 if general else bf16
    # fast path folds seq' = -2*seq into w_q (scores) and the fc fold (values)
    qscale = 1.0 if general else -0.5
    # fast path: z-gate tanh(gz/2) folded into wihzT; W_v folded into fc:
    #   fc(attn@v) = (attn@seq') @ (fc_w_h @ W_v_h)^T * (-0.5) per head
    if general:
        fcw0 = fc_w.T * (np.arange(128) < 64)[:, None].astype(f32)
        fcw1 = fc_w.T * (np.arange(128) >= 64)[:, None].astype(f32)
        zsc = 1.0
    else:
        fcw0 = (fc_w[:, 0:64] @ w_vs[0:64, :]).T * -0.5
        fcw1 = (fc_w[:, 64:128] @ w_vs[64:128, :]).T * -0.5
        zsc = 0.5
    com = {
        "eye": np.eye(128, dtype=f32),
        "ones2": np.concatenate([np.ones((128, 1), f32), np.zeros((128, 1), f32)], 1).astype(wdt),
        "onesrow": np.ones((1, 128), wdt),
        "wqT": np.ascontiguousarray((w_qs / TEMP).T * qscale).astype(wdt),
        "wks": np.ascontiguousarray(w_ks).astype(wdt),
        "wihzT": np.ascontiguousarray(w_ih[128:256].T * zsc).astype(wdt),
        "wihnT": np.ascontiguousarray(w_ih[256:384].T).astype(wdt),
        "fcw0T": np.ascontiguousarray(fcw0).astype(wdt),
        "fcw1T": np.ascontiguousarray(fcw1).astype(wdt),
        "m1aT": np.ascontiguousarray(m_fc1[:, :128].T).astype(wdt),
        "m1bT": np.ascontiguousarray(m_fc1[:, 128:].T).astype(wdt),
        "m2T": np.ascontiguousarray(m_fc2.T).astype(wdt),
        "fc_b": np.asarray(inputs["fc_b"], f32).reshape(128, 1),
        "ln_g": np.asarray(inputs["ln_g"], f32).reshape(128, 1),
        "ln_b": np.asarray(inputs["ln_b"], f32).reshape(128, 1),
        "m1b": np.asarray(inputs["m_fc1_b"], f32).reshape(128, 1),
        "m2b": np.asarray(inputs["m_fc2_b"], f32).reshape(128, 1),
    }
    if general:
        com["zeros128"] = np.zeros((128, 128), f32)
        com["wvsT"] = np.ascontiguousarray(w_vs.T).astype(f32)
        w_hh = np.asarray(inputs["gru_w_hh"], f32)
        b_ih = np.asarray(inputs["gru_b_ih"], f32)
        b_hh = np.asarray(inputs["gru_b_hh"], f32)
        com.update({
            "wihrT": np.ascontiguousarray(w_ih[0:128].T),
            "whhrT": np.ascontiguousarray(w_hh[0:128].T),
            "whhzT": np.ascontiguousarray(w_hh[128:256].T),
            "whhnT": np.ascontiguousarray(w_hh[256:384].T),
            "b_r": (b_ih[0:128] + b_hh[0:128]).reshape(128, 1).astype(f32),
            "b_z": (b_ih[128:256] + b_hh[128:256]).reshape(128, 1).astype(f32),
            "b_in": b_ih[256:384].reshape(128, 1).astype(f32),
            "b_hn": b_hh[256:384].reshape(128, 1).astype(f32),
        })
    else:
        com["eyebf"] = np.eye(128, dtype=f32).astype(bf16)

    # additive mask, per core: [N_ST, 128(=32ac+2g+h), 1024] (bf16)
    m3 = mask.reshape(N_CORES, B_CORE, N_SRC, NN)  # [core, b, s, n]
    st_i = np.arange(N_ST)
    cc_i = np.arange(4)
    g_i = np.arange(16)
    b_idx = 2 * st_i[:, None] + cc_i[None, :] // 2          # [st, cc]
    s_idx = (cc_i[:, None] % 2) * 16 + g_i[None, :]         # [cc, g]
    # fast path: score column c of any ac-block maps to within-block row
    # perm[c] = ((c%128)//8)*64 + (c%8)*8 + c//128  (4KB-segment x layout)
    c_arr = np.arange(1024)
    col_perm = ((c_arr % 128) // 8) * 64 + (c_arr % 8) * 8 + c_arr // 128
    maskfull_cores = []
    for core in range(N_CORES):
        msel = m3[core][b_idx[:, :, None], s_idx[None, :, :]]   # [st, cc, g, 64]
        vals = np.where(msel, f32(NEG_INF), f32(0.0))           # [st, cc, g, 64]
        out = np.full((N_ST, 4, 16, 2, 16, 64), NEG_INF, f32)
        out[:, :, g_i, :, g_i, :] = vals.transpose(2, 0, 1, 3)[:, :, :, None, :]
        out = out.reshape(N_ST, 128, 1024)
        if not general:
            out = out[:, :, col_perm]
        maskfull_cores.append(np.ascontiguousarray(out).astype(bf16))

    in_maps = []
    hid = None
    if general:
        hid = np.ascontiguousarray(np.asarray(inputs["hidden"], f32))
    for core in range(N_CORES):
        m = dict(com)
        m["ngh"] = ngh[core * ROWS:(core + 1) * ROWS]
        m["srcf"] = src[core * B_CORE:(core + 1) * B_CORE].reshape(B_CORE * N_SRC, D)
        m["maskfull"] = maskfull_cores[core]
        if general:
            m["hid"] = hid[core * ROWS:(core + 1) * ROWS]
        in_maps.append(m)
    return in_maps


def _get_program(general, tune=None):
    key = (general, tuple(sorted((tune or {}).items())))
    if key not in _PROG_CACHE:
        if general:
            _PROG_CACHE[key] = build_program_v1(True)
        else:
            _PROG_CACHE[key] = build_program_fast(tune)
    return _PROG_CACHE[key]


def _is_fast_path(inputs):
    if np.asarray(inputs["gru_b_ih"]).any() or np.asarray(inputs["gru_b_hh"]).any():
        return False
    return not np.asarray(inputs["hidden"]).any()


def run(inputs, trace=False, force_general=None, tune=None):
    if tune is None and os.environ.get("K_TUNE"):
        tune = dict(kv.split("=") for kv in os.environ["K_TUNE"].split(","))
        tune = {k: int(v) for k, v in tune.items()}
    from concourse.bass_utils import run_bass_kernel_spmd
    general = (not _is_fast_path(inputs)) if force_general is None else force_general
    nc = _get_program(general, tune)
    in_maps = _prep_inputs(inputs, general)
    res = run_bass_kernel_spmd(nc, in_maps, list(range(N_CORES)), trace=trace)
    z = np.stack([r["z"] for r in res.results], axis=0)  # [8, 1024, 128]
    out = z.reshape(N_CORES, B_CORE, N_SRC, D).reshape(B, N_SRC, D).astype(np.float32)
    return out, res


def kernel(**inputs) -> np.ndarray:
    out, _ = run(inputs, trace=False)
    return out


# revision 35
# speedup vs baseline: 1.1380x; 1.1380x over previous
"""Trainium2 Bass kernel for nn_CAWN2 (CAWN-style GNN message passing).

Reference computation (per full input):
  seq = GRUCell(ngh_feat, hidden)                      # [B*2048, 128]
  grouped 2-head attention: q from src, k/v from seq,
  64 neighbors per (b, s) group, additive -1e10 mask,
  softmax, out proj, residual + LayerNorm, 2-layer MLP  -> [B, 32, 128]

Strategy: data-parallel over batch across 8 NeuronCores (32 batches/core).
Per core, a feature-major bf16 pipeline processes 16 supertiles of 4096
neighbor rows (2 batches):
  - PE transposes x into [d, n] tiles; copies cast to bf16 on DVE/ACT
  - fast path (hidden==0, biases==0) with tanh-only gates:
      seq' = (tanh(gz/2) - 1) * tanh(gn) = -2 * sigmoid(-gz)*tanh(gn)
    the -0.5 factor is folded into w_q and w_vs host-side, so the whole
    kernel uses one ACT table set (exp_and_others) -- no table switches
  - w_ks folded into q; scores via column-tiled matmuls with [128,32]
    stationaries (4 PE column groups), host-built additive bf16 mask
  - exp with accum_out row sums; 1/sum normalization on bf16 at 4x DVE
  - attn@v as v_j^T @ attn_j^T accumulating the output transpose directly
  - residual + LayerNorm + merge MLP batched over all 16 supertiles in
    two 512-row segments; rstd via ACT Rsqrt (one table switch at the end)
"""

import os
import sys
from contextlib import ExitStack

import numpy as np

sys.path.insert(0, "/opt/trn_rl_repo")

import ml_dtypes  # noqa: E402

import concourse.bass as bass  # noqa: E402
import concourse.bacc as bacc  # noqa: E402
import concourse.mybir as mybir  # noqa: E402
import concourse.tile as tile  # noqa: E402

F32 = mybir.dt.float32
F32R = mybir.dt.float32r
BF16 = mybir.dt.bfloat16
I32 = mybir.dt.int32
AF = mybir.ActivationFunctionType
ALU = mybir.AluOpType
AX = mybir.AxisListType

N_CORES = 8
B, N_SRC, N_NGH, D, H = 256, 32, 2048, 128, 2
DK = D // H
NN = N_NGH // N_SRC  # 64 neighbors per group
NEG_INF = -1e10
LN_EPS = 1e-5
TEMP = float(np.sqrt(DK))  # 8.0

B_CORE = B // N_CORES          # 32 batches per core
ROWS = B_CORE * N_NGH          # 65536 neighbor rows per core
SRC_ROWS = B_CORE * N_SRC      # 1024 src rows per core
ST_ROWS = 4096                 # supertile = 2 batches
N_ST = ROWS // ST_ROWS         # 16
N_GC = 8                       # 512-row GRU chunks per supertile
N_AC = 4                       # 1024-row attention chunks per supertile

_PROG_CACHE: dict = {}


def build_program_fast(tune: dict | None = None):
    """bf16 fast-path program (hidden==0, gru biases==0)."""
    tn = {"seq": 3, "x": 3, "big": 3, "med": 3, "sm": 2, "gp_stt": 1,
          "xtd_mod": 2, "pe_mask": 1, "gru1024": 1}
    tn.update(tune or {})
    nc = bacc.Bacc("TRN2")

    # ---- DRAM I/O ----
    t_ngh = nc.dram_tensor("ngh", [ROWS, D], F32, kind="ExternalInput")
    t_src = nc.dram_tensor("srcf", [SRC_ROWS, D], F32, kind="ExternalInput")
    t_mask = nc.dram_tensor("maskfull", [N_ST, 128, 1024], BF16, kind="ExternalInput")
    t_eye = nc.dram_tensor("eye", [128, 128], F32, kind="ExternalInput")
    # bf16 consts packed into one tensor: 9 weights + eyebf + [ones2|onesrow pad]
    wnames = ["wqT", "wks", "wihzT", "wihnT",
              "fcw0T", "fcw1T", "m1aT", "m1bT", "m2T"]
    NW = len(wnames)
    t_wpack = nc.dram_tensor("wpack", [128, (NW + 1) * 128 + 4], BF16,
                             kind="ExternalInput")
    vnames = ["fc_b", "ln_g", "ln_b", "m1b", "m2b"]
    t_vpack = nc.dram_tensor("vpack", [128, len(vnames)], F32, kind="ExternalInput")
    t_onesr = nc.dram_tensor("onesrow", [1, 128], BF16, kind="ExternalInput")
    t_out = nc.dram_tensor("z", [SRC_ROWS, D], F32, kind="ExternalOutput")

    # supertile rows laid out so each DMA partition reads 4 contiguous 4KB
    # segments: supertile position (p, t) holds DRAM row
    #   d = (t//8)*1024 + (p//8)*64 + (p%8)*8 + (t%8)
    # which keeps each neighbor group inside one score ac-block; the host
    # permutes mask columns to match.
    ngh_v = t_ngh[:, :].rearrange("(st ac k i j) d -> st (k i) ac j d",
                                  st=N_ST, ac=4, k=16, i=8, j=8)
    src_v = t_src[:, :].rearrange("(t p) d -> p t d", t=8, p=128)
    out_v = t_out[:, :].rearrange("(t p) d -> p t d", t=8, p=128)

    with tile.TileContext(nc) as tc, ExitStack() as ctx:
        consts = ctx.enter_context(tc.tile_pool(name="consts", bufs=1))
        p_seq = ctx.enter_context(tc.tile_pool(name="p_seq", bufs=tn["seq"]))
        p_x = ctx.enter_context(tc.tile_pool(name="p_x", bufs=tn["x"]))
        p_big = ctx.enter_context(tc.tile_pool(name="p_big", bufs=tn["big"]))
        p_med = ctx.enter_context(tc.tile_pool(name="p_med", bufs=tn["med"]))
        p_sm = ctx.enter_context(tc.tile_pool(name="p_sm", bufs=tn["sm"]))
        p_keep = ctx.enter_context(tc.tile_pool(name="p_keep", bufs=1))
        # PSUM: 8 banks total
        ps_xt = ctx.enter_context(tc.tile_pool(name="ps_xt", bufs=2, space="PSUM"))
        ps_gi = ctx.enter_context(tc.tile_pool(
            name="ps_gi", bufs=1 if tn["gru1024"] else 2, space="PSUM"))
        ps_sc = ctx.enter_context(tc.tile_pool(name="ps_sc", bufs=1, space="PSUM"))
        ps_v = ctx.enter_context(tc.tile_pool(name="ps_v", bufs=1, space="PSUM"))
        ps_sm = ctx.enter_context(tc.tile_pool(name="ps_sm", bufs=1, space="PSUM"))

        # ---- load constants (3 packed DMAs + 1 row) ----
        eye = consts.tile([128, 128], F32)
        nc.sync.dma_start(out=eye, in_=t_eye[:, :])
        sb_wpack = consts.tile([128, (NW + 1) * 128 + 4], BF16, name="wpack")
        nc.sync.dma_start(out=sb_wpack, in_=t_wpack[:, :])
        w_sb = {n: sb_wpack[:, i * 128:(i + 1) * 128] for i, n in enumerate(wnames)}
        eyebf = sb_wpack[:, NW * 128:(NW + 1) * 128]
        ones2 = sb_wpack[:, (NW + 1) * 128:(NW + 1) * 128 + 2]
        sb_vpack = consts.tile([128, len(vnames)], F32, name="vpack")
        nc.sync.dma_start(out=sb_vpack, in_=t_vpack[:, :])
        v_sb = {n: sb_vpack[:, i:i + 1] for i, n in enumerate(vnames)}
        ones_row = consts.tile([1, 128], BF16)
        nc.sync.dma_start(out=ones_row, in_=t_onesr[:, :])

        def transpose(out_ap, in_ap, ident, base=0, k=128):
            tp = (base, 0) if base else None
            nc.tensor.transpose(out_ap, in_ap, ident[base:base + k, base:base + k],
                                tile_position=tp)

        # ---- precompute: srcT (bf16) and q for all supertiles ----
        sb_src_rm = p_keep.tile([128, 8, 128], BF16, tag="src_rm")
        nc.gpsimd.dma_start(out=sb_src_rm, in_=src_v)
        sb_srcT = p_keep.tile([128, SRC_ROWS], BF16, tag="srcT")
        for r in range(2):
            pt_st = ps_xt.tile([128, 512], BF16, tag="xt", name=f"pt_src{r}")
            for t4 in range(4):
                transpose(pt_st[:, t4 * 128:(t4 + 1) * 128],
                          sb_src_rm[:, r * 4 + t4, :], eyebf)
            nc.vector.tensor_copy(out=sb_srcT[:, r * 512:(r + 1) * 512], in_=pt_st)
        sb_q = p_keep.tile([128, SRC_ROWS], BF16, tag="q_all")
        pt_q = ps_sc.tile([128, 1024], F32, tag="sc", name="pt_q")
        nc.tensor.matmul(pt_q[:, 0:512], w_sb["wqT"], sb_srcT[:, 0:512],
                         start=True, stop=True)
        nc.tensor.matmul(pt_q[:, 512:1024], w_sb["wqT"], sb_srcT[:, 512:1024],
                         start=True, stop=True)
        nc.vector.tensor_copy(out=sb_q, in_=pt_q)

        sb_oaT_all = p_keep.tile([128, N_ST * 128], BF16, tag="oaT_all")

        def gru_phase(st):
            sb_seqT = p_seq.tile([128, ST_ROWS], BF16, tag="seqT")
            x_bf = p_x.tile([128, 32, 128], BF16, tag="x_bf")
            xv = x_bf[:, :, :].rearrange("p (ac j) d -> p ac j d", ac=4)
            nc.gpsimd.dma_start(out=xv, in_=ngh_v[st])
            for gc in range(N_GC):
                pt_xt = ps_xt.tile([128, 512], BF16, tag="xt")
                for t in range(4):
                    transpose(pt_xt[:, t * 128:(t + 1) * 128],
                              x_bf[:, gc * 4 + t, :], eyebf)
                sb_xT = p_med.tile([128, 512], BF16, tag="xT")
                if (st * N_GC + gc) % tn["xtd_mod"] == 0:
                    nc.scalar.copy(out=sb_xT, in_=pt_xt)
                else:
                    nc.vector.tensor_copy(out=sb_xT, in_=pt_xt)
                if tn["gru1024"]:
                    # 0.5 for the z-gate is folded into wihzT host-side, so
                    # one tanh covers both gates in a 2-bank PSUM tile
                    pt_g = ps_gi.tile([128, 1024], F32, tag="gi", name="pt_g")
                    nc.tensor.matmul(pt_g[:, 0:512], w_sb["wihzT"], sb_xT,
                                     start=True, stop=True)
                    nc.tensor.matmul(pt_g[:, 512:1024], w_sb["wihnT"], sb_xT,
                                     start=True, stop=True)
                    sb_tab = p_med.tile([128, 1024], BF16, tag="tab")
                    nc.scalar.activation(out=sb_tab, in_=pt_g, func=AF.Tanh)
                    sb_ta = sb_tab[:, 0:512]
                    sb_tb = sb_tab[:, 512:1024]
                else:
                    pt_gz = ps_gi.tile([128, 512], F32, tag="gi", name="pt_gz")
                    nc.tensor.matmul(pt_gz, w_sb["wihzT"], sb_xT, start=True, stop=True)
                    pt_gn = ps_gi.tile([128, 512], F32, tag="gi", name="pt_gn")
                    nc.tensor.matmul(pt_gn, w_sb["wihnT"], sb_xT, start=True, stop=True)
                    sb_ta = p_med.tile([128, 512], BF16, tag="ta")
                    nc.scalar.activation(out=sb_ta, in_=pt_gz, func=AF.Tanh, scale=0.5)
                    sb_tb = p_med.tile([128, 512], BF16, tag="tb")
                    nc.scalar.activation(out=sb_tb, in_=pt_gn, func=AF.Tanh)
                # seq' = (ta - 1) * tb  (= -2 * sigmoid(-gz) * tanh(gn))
                if tn["gp_stt"] and gc % 2 == 1:
                    # offload to GpSimd as mul+sub (STT unsupported on Pool)
                    sb_p = p_med.tile([128, 512], BF16, tag="gp_p")
                    nc.gpsimd.tensor_mul(sb_p, sb_ta, sb_tb)
                    nc.gpsimd.tensor_sub(sb_seqT[:, gc * 512:(gc + 1) * 512],
                                         sb_p, sb_tb)
                else:
                    nc.vector.scalar_tensor_tensor(
                        out=sb_seqT[:, gc * 512:(gc + 1) * 512],
                        in0=sb_ta, scalar=1.0, in1=sb_tb,
                        op0=ALU.subtract, op1=ALU.mult)
            # q' = wks^T @ qemb (head-embedded q columns for this supertile)
            sb_qemb = p_sm.tile([128, 128], BF16, tag="qemb")
            nc.gpsimd.memset(sb_qemb, 0.0)
            qe_v = sb_qemb[:, :].rearrange("p (cg h) -> p h cg", h=2)
            nc.vector.tensor_copy(out=qe_v[0:64, 0, :],
                                  in_=sb_q[0:64, st * 64:(st + 1) * 64])
            nc.vector.tensor_copy(out=qe_v[64:128, 1, :],
                                  in_=sb_q[64:128, st * 64:(st + 1) * 64])
            pt_qp = ps_v.tile([128, 128], F32, tag="v", name="pt_qp")
            nc.tensor.matmul(pt_qp, w_sb["wks"], sb_qemb, start=True, stop=True)
            sb_qp = p_sm.tile([128, 128], BF16, tag="qp", bufs=3)
            nc.vector.tensor_copy(out=sb_qp, in_=pt_qp)
            return sb_seqT, sb_qp

        def attn_phase(st, sb_seqT, sb_qp):
            sb_mask = p_big.tile([128, 1024], BF16, tag="mask")
            nc.sync.dma_start(out=sb_mask, in_=t_mask[st])

            # column-tiled scores: partitions 32ac..32ac+32 <- q'_ac x seqT_ac
            # the additive mask is pre-accumulated into PSUM via an identity
            # matmul, so exp can read PSUM directly (no DVE mask add)
            pt_sc = ps_sc.tile([128, 1024], F32, tag="sc")
            pe_mask = tn["pe_mask"]
            if pe_mask:
                for half in range(2):
                    nc.tensor.matmul(
                        pt_sc[:, half * 512:(half + 1) * 512],
                        eyebf, sb_mask[:, half * 512:(half + 1) * 512],
                        start=True, stop=False, skip_group_check=True)
            for ac in range(N_AC):
                for half in range(2):
                    nc.tensor.matmul(
                        pt_sc[32 * ac:32 * ac + 32, half * 512:(half + 1) * 512],
                        sb_qp[:, 32 * ac:32 * ac + 32],
                        sb_seqT[:, ac * 1024 + half * 512:
                                ac * 1024 + (half + 1) * 512],
                        start=not pe_mask, stop=True,
                        tile_position=(0, 32 * ac), skip_group_check=True)

            sb_attn = p_big.tile([128, 1024], BF16, tag="attn")
            sb_sums = p_sm.tile([128, 1], F32, tag="sums")
            if pe_mask:
                nc.scalar.activation(out=sb_attn, in_=pt_sc, func=AF.Exp,
                                     accum_out=sb_sums)
            else:
                sb_scm = p_big.tile([128, 1024], BF16, tag="scm")
                nc.vector.tensor_add(sb_scm, pt_sc, sb_mask)
                nc.scalar.activation(out=sb_attn, in_=sb_scm, func=AF.Exp,
                                     accum_out=sb_sums)
            sb_sum2 = p_sm.tile([128, 1], F32, tag="sums", name="sb_sum2")
            nc.vector.tensor_scalar_add(sb_sum2, sb_sums, 1e-30)
            sb_rec = p_sm.tile([128, 1], F32, tag="rec")
            nc.vector.reciprocal(sb_rec, sb_sum2)
            sb_attn_n = p_big.tile([128, 1024], BF16, tag="attn_n")
            nc.vector.tensor_scalar_mul(sb_attn_n, sb_attn, sb_rec)

            pt_oaT = ps_sm.tile([128, 128], F32, tag="sm", name="pt_oaT")
            for ac in range(N_AC):
                base = ac * 1024
                # row-major seq chunks via PE transpose; W_v is folded into
                # the fc weights host-side, so oaT = sum_j seq_j^T-rows @ at_j
                sb_vrm = p_sm.tile([128, 8, 128], BF16, tag="vrm", bufs=2)
                for half in range(2):
                    pt_v = ps_v.tile([128, 512], BF16, tag="v")
                    for j in range(4):
                        sl = sb_seqT[:, base + half * 512 + j * 128:
                                     base + half * 512 + (j + 1) * 128]
                        transpose(pt_v[:, j * 128:(j + 1) * 128], sl, eyebf)
                    nc.vector.tensor_copy(out=sb_vrm[:, half * 4:(half + 1) * 4, :],
                                          in_=pt_v)
                pt_at = ps_v.tile([128, 256], BF16, tag="v", name="pt_at")
                for j in range(8):
                    transpose(pt_at[:, j * 32:(j + 1) * 32],
                              sb_attn_n[32 * ac:32 * ac + 32,
                                        j * 128:(j + 1) * 128],
                              eyebf, base=32 * ac, k=32)
                sb_at = p_sm.tile([128, 256], BF16, tag="at")
                nc.vector.tensor_copy(out=sb_at, in_=pt_at)
                # oaT[:, 32ac:32ac+32] = sum_j seq_j^T @ at_j   ([feat, slot])
                for j in range(8):
                    nc.tensor.matmul(pt_oaT[:, 32 * ac:32 * ac + 32],
                                     sb_vrm[:, j, :],
                                     sb_at[:, j * 32:(j + 1) * 32],
                                     start=(j == 0), stop=(j == 7))
            nc.vector.tensor_copy(out=sb_oaT_all[:, st * 128:(st + 1) * 128],
                                  in_=pt_oaT)

        # ---- batched tail: fc + residual + LN + merge MLP, 2 segments ----
        oa_v = sb_oaT_all[:, :].rearrange("p (r h) -> p h r", h=2)

        def tail_seg(seg):
            rsl = slice(seg * 512, (seg + 1) * 512)
            pt_fc = ps_sc.tile([128, 512], F32, tag="sc", name="pt_fc")
            nc.tensor.matmul(pt_fc, w_sb["fcw0T"], oa_v[:, 0, rsl],
                             start=True, stop=False)
            nc.tensor.matmul(pt_fc, w_sb["fcw1T"], oa_v[:, 1, rsl],
                             start=False, stop=True)
            sb_x1 = p_med.tile([128, 512], BF16, tag="x1")
            nc.vector.tensor_scalar_add(sb_x1, pt_fc, v_sb["fc_b"])
            sb_x2 = p_med.tile([128, 512], BF16, tag="x2", bufs=2)
            nc.vector.tensor_add(sb_x2, sb_x1, sb_srcT[:, rsl])
            sb_sq = p_med.tile([128, 512], BF16, tag="sq")
            nc.scalar.activation(out=sb_sq, in_=sb_x2, func=AF.Square)
            pt_ln = ps_sc.tile([2, 1024], F32, tag="sc", name="pt_ln")
            nc.tensor.matmul(pt_ln[0:2, 0:512], ones2, sb_x2,
                             start=True, stop=True)
            nc.tensor.matmul(pt_ln[0:2, 512:1024], ones2, sb_sq,
                             start=True, stop=True)
            sb_mu = p_sm.tile([1, 512], F32, tag="mu")
            nc.vector.tensor_scalar_mul(sb_mu, pt_ln[0:1, 0:512], 1.0 / 128.0)
            sb_ex2 = p_sm.tile([1, 512], F32, tag="ex2")
            nc.vector.tensor_scalar(sb_ex2, pt_ln[0:1, 512:1024],
                                    1.0 / 128.0, LN_EPS,
                                    op0=ALU.mult, op1=ALU.add)
            sb_musq = p_sm.tile([1, 512], F32, tag="musq")
            nc.vector.tensor_mul(sb_musq, sb_mu, sb_mu)
            sb_ve = p_sm.tile([1, 512], F32, tag="ve")
            nc.vector.tensor_sub(sb_ve, sb_ex2, sb_musq)
            sb_stats = p_sm.tile([1, 1024], BF16, tag="stats")
            nc.vector.tensor_copy(out=sb_stats[0:1, 0:512], in_=sb_mu)
            sb_rve = p_sm.tile([1, 512], F32, tag="rve")
            nc.vector.reciprocal(sb_rve, sb_ve)
            nc.scalar.activation(out=sb_stats[0:1, 512:1024], in_=sb_rve,
                                 func=AF.Sqrt)
            pt_bc = ps_sc.tile([128, 1024], F32, tag="sc", name="pt_bc")
            nc.tensor.matmul(pt_bc[:, 0:512], ones_row, sb_stats[0:1, 0:512],
                             start=True, stop=True)
            nc.tensor.matmul(pt_bc[:, 512:1024], ones_row,
                             sb_stats[0:1, 512:1024], start=True, stop=True)
            sb_xc = p_med.tile([128, 512], BF16, tag="xc")
            nc.vector.tensor_sub(sb_xc, sb_x2, pt_bc[:, 0:512])
            sb_xn0 = p_med.tile([128, 512], BF16, tag="xn0")
            nc.vector.tensor_mul(sb_xn0, sb_xc, pt_bc[:, 512:1024])
            sb_xn = p_med.tile([128, 512], BF16, tag="xn")
            nc.vector.tensor_scalar(sb_xn, sb_xn0, v_sb["ln_g"], v_sb["ln_b"],
                                    op0=ALU.mult, op1=ALU.add)
            pt_h1 = ps_gi.tile([128, 512], F32, tag="gi", name="pt_h1")
            nc.tensor.matmul(pt_h1, w_sb["m1aT"], sb_xn, start=True, stop=False)
            nc.tensor.matmul(pt_h1, w_sb["m1bT"], sb_srcT[:, rsl],
                             start=False, stop=True)
            sb_h1 = p_med.tile([128, 512], BF16, tag="h1")
            nc.scalar.activation(out=sb_h1, in_=pt_h1, func=AF.Relu,
                                 bias=v_sb["m1b"])
            pt_z = ps_gi.tile([128, 512], F32, tag="gi", name="pt_z")
            nc.tensor.matmul(pt_z, w_sb["m2T"], sb_h1, start=True, stop=True)
            sb_zb = p_med.tile([128, 512], F32, tag="zb")
            nc.vector.tensor_scalar_add(sb_zb, pt_z, v_sb["m2b"])
            pt_zr = ps_xt.tile([128, 512], F32, tag="xt", name="pt_zr")
            for t in range(4):
                transpose(pt_zr[:, t * 128:(t + 1) * 128],
                          sb_zb[:, t * 128:(t + 1) * 128], eye)
            sb_zout = p_med.tile([128, 4, 128], F32, tag="zout")
            nc.vector.tensor_copy(out=sb_zout, in_=pt_zr)
            nc.sync.dma_start(out=out_v[:, seg * 4:(seg + 1) * 4, :], in_=sb_zout)

        for st in range(N_ST):
            carry = gru_phase(st)
            attn_phase(st, *carry)
        tail_seg(0)
        tail_seg(1)

    nc.finalize()
    return nc


# ----------------------------------------------------------------------------
# v1 general-path program (f32r, handles nonzero hidden / gru biases)
# ----------------------------------------------------------------------------

def build_program_v1(general: bool, use_f32r: bool = True, xt_copy: str = "act"):
    tn = {"seq": 5, "stl": 5, "chunk": 3, "att": 2, "sm": 3,
          "xt": 1, "gi": 2, "v": 1, "sc": 1, "psm": 2}
    nc = bacc.Bacc("TRN2")
    MMDT = F32R if use_f32r else F32

    t_ngh = nc.dram_tensor("ngh", [ROWS, D], F32, kind="ExternalInput")
    t_src = nc.dram_tensor("srcf", [B_CORE * N_SRC, D], F32, kind="ExternalInput")
    t_mask = nc.dram_tensor("maskfull", [N_ST, 128, 1024], BF16, kind="ExternalInput")
    t_eye = nc.dram_tensor("eye", [128, 128], F32, kind="ExternalInput")
    wnames = ["wqT", "wks", "wihzT", "wihnT", "wvsT",
              "fcw0T", "fcw1T", "m1aT", "m1bT", "m2T"]
    if general:
        wnames += ["wihrT", "whhrT", "whhzT", "whhnT"]
    t_w = {n: nc.dram_tensor(n, [128, 128], MMDT, kind="ExternalInput") for n in wnames}
    t_onesc = nc.dram_tensor("ones2", [128, 2], MMDT, kind="ExternalInput")
    t_onesr = nc.dram_tensor("onesrow", [1, 128], MMDT, kind="ExternalInput")
    t_zeros = nc.dram_tensor("zeros128", [128, 128], MMDT, kind="ExternalInput")
    vnames = ["fc_b", "ln_g", "ln_b", "m1b", "m2b"]
    if general:
        vnames += ["b_r", "b_z", "b_in", "b_hn"]
    t_v = {n: nc.dram_tensor(n, [128, 1], F32, kind="ExternalInput") for n in vnames}
    if general:
        t_hid = nc.dram_tensor("hid", [ROWS, D], F32, kind="ExternalInput")
    t_out = nc.dram_tensor("z", [B_CORE * N_SRC, D], F32, kind="ExternalOutput")

    ngh_v = t_ngh[:, :].rearrange("(st gc t p) d -> st gc p t d", st=N_ST, gc=N_GC, t=4, p=128)
    if general:
        hid_v = t_hid[:, :].rearrange("(st gc t p) d -> st gc p t d", st=N_ST, gc=N_GC, t=4, p=128)

    with tile.TileContext(nc) as tc, ExitStack() as ctx:
        consts = ctx.enter_context(tc.tile_pool(name="consts", bufs=1))
        p_seq = ctx.enter_context(tc.tile_pool(name="p_seq", bufs=tn["seq"]))
        p_stl = ctx.enter_context(tc.tile_pool(name="p_stl", bufs=tn["stl"]))
        p_chunk = ctx.enter_context(tc.tile_pool(name="p_chunk", bufs=tn["chunk"]))
        p_att = ctx.enter_context(tc.tile_pool(name="p_att", bufs=tn["att"]))
        p_sm = ctx.enter_context(tc.tile_pool(name="p_sm", bufs=tn["sm"]))
        ps_xt = ctx.enter_context(tc.tile_pool(name="ps_xt", bufs=tn["xt"], space="PSUM"))
        ps_gi = ctx.enter_context(tc.tile_pool(name="ps_gi", bufs=tn["gi"], space="PSUM"))
        ps_v = ctx.enter_context(tc.tile_pool(name="ps_v", bufs=tn["v"], space="PSUM"))
        ps_sc = ctx.enter_context(tc.tile_pool(name="ps_sc", bufs=tn["sc"], space="PSUM"))
        ps_at = ctx.enter_context(tc.tile_pool(name="ps_at", bufs=1, space="PSUM"))
        ps_sm = ctx.enter_context(tc.tile_pool(name="ps_sm", bufs=tn["psm"], space="PSUM"))

        eye = consts.tile([128, 128], F32)
        nc.sync.dma_start(out=eye, in_=t_eye[:, :])
        w_sb = {}
        for n in wnames:
            w_sb[n] = consts.tile([128, 128], MMDT, name=f"w_{n}")
            nc.sync.dma_start(out=w_sb[n], in_=t_w[n][:, :])
        v_sb = {}
        for n in vnames:
            v_sb[n] = consts.tile([128, 1], F32, name=f"v_{n}")
            nc.sync.dma_start(out=v_sb[n], in_=t_v[n][:, :])
        ones2 = consts.tile([128, 2], MMDT)
        nc.sync.dma_start(out=ones2, in_=t_onesc[:, :])
        zconst = consts.tile([128, 128], MMDT)
        nc.sync.dma_start(out=zconst, in_=t_zeros[:, :])
        ones_row = consts.tile([1, 128], MMDT)
        nc.sync.dma_start(out=ones_row, in_=t_onesr[:, :])

        def transpose(out_ap, in_ap, base=0, k=128):
            tp = (base, 0) if base else None
            nc.tensor.transpose(out_ap, in_ap, eye[base:base + k, base:base + k],
                                tile_position=tp)

        def gru_phase(st):
            sb_src = p_sm.tile([64, 128], F32, tag="src")
            nc.sync.dma_start(out=sb_src, in_=t_src[st * 64:(st + 1) * 64, :])
            pt_srcT = ps_sm.tile([128, 128], F32, tag="psmall", name="pt_srcT")
            transpose(pt_srcT[:, 0:64], sb_src, k=64)
            sb_srcT = p_stl.tile([128, 64], MMDT, tag="srcT")
            nc.scalar.copy(out=sb_srcT, in_=pt_srcT[:, 0:64])

            pt_q = ps_sm.tile([128, 128], F32, tag="psmall", name="pt_q")
            nc.tensor.matmul(pt_q[:, 0:64], w_sb["wqT"],
                             sb_srcT, start=True, stop=True)
            sb_qT = p_sm.tile([128, 64], F32, tag="qT")
            nc.vector.tensor_copy(out=sb_qT, in_=pt_q[:, 0:64])
            sb_qemb = p_sm.tile([128, 128], MMDT, tag="qemb")
            nc.sync.dma_start(out=sb_qemb, in_=t_zeros[:, :])
            qe_v = sb_qemb[:, :].rearrange("p (cg h) -> p h cg", h=2)
            nc.vector.tensor_copy(out=qe_v[0:64, 0, :], in_=sb_qT[0:64, :])
            nc.vector.tensor_copy(out=qe_v[64:128, 1, :], in_=sb_qT[64:128, :])
            pt_qp = ps_sm.tile([128, 128], F32, tag="psmall", name="pt_qp")
            nc.tensor.matmul(pt_qp, w_sb["wks"],
                             sb_qemb, start=True, stop=True)
            sb_qp = []
            for c in range(N_AC):
                qz = p_stl.tile([128, 128], MMDT, tag="qpz", bufs=8, name=f"qz{c}")
                nc.sync.dma_start(out=qz, in_=zconst)
                nc.vector.tensor_copy(out=qz[:, 32 * c:32 * c + 32],
                                      in_=pt_qp[:, 32 * c:32 * c + 32])
                sb_qp.append(qz)

            sb_seqT = p_seq.tile([128, ST_ROWS], MMDT, tag="seqT")
            for gc in range(N_GC):
                x_rm = p_chunk.tile([128, 4, 128], F32, tag="x_rm")
                nc.sync.dma_start(out=x_rm, in_=ngh_v[st, gc])
                pt_xt = ps_xt.tile([128, 512], F32, tag="xt")
                for t in range(4):
                    transpose(pt_xt[:, t * 128:(t + 1) * 128], x_rm[:, t, :])
                sb_xT = p_chunk.tile([128, 512], MMDT, tag="xT")
                nc.scalar.copy(out=sb_xT, in_=pt_xt)
                h_rm = p_chunk.tile([128, 4, 128], F32, tag="h_rm")
                nc.sync.dma_start(out=h_rm, in_=hid_v[st, gc])
                pt_ht = ps_xt.tile([128, 512], F32, tag="xt", name="pt_ht")
                for t in range(4):
                    transpose(pt_ht[:, t * 128:(t + 1) * 128], h_rm[:, t, :])
                sb_hT = p_chunk.tile([128, 512], MMDT, tag="hT")
                nc.scalar.copy(out=sb_hT, in_=pt_ht)

                seq_sl = sb_seqT[:, gc * 512:(gc + 1) * 512]
                pt_gr = ps_gi.tile([128, 512], F32, tag="gi", name="pt_gr")
                nc.tensor.matmul(pt_gr, w_sb["wihrT"],
                                 sb_xT, start=True, stop=False)
                nc.tensor.matmul(pt_gr, w_sb["whhrT"],
                                 sb_hT, start=False, stop=True)
                pt_gz = ps_gi.tile([128, 512], F32, tag="gi", name="pt_gz")
                nc.tensor.matmul(pt_gz, w_sb["wihzT"],
                                 sb_xT, start=True, stop=False)
                nc.tensor.matmul(pt_gz, w_sb["whhzT"],
                                 sb_hT, start=False, stop=True)
                pt_gni = ps_gi.tile([128, 512], F32, tag="gi", name="pt_gni")
                nc.tensor.matmul(pt_gni, w_sb["wihnT"],
                                 sb_xT, start=True, stop=True)
                pt_gnh = ps_gi.tile([128, 512], F32, tag="gi", name="pt_gnh")
                nc.tensor.matmul(pt_gnh, w_sb["whhnT"],
                                 sb_hT, start=True, stop=True)
                sb_r = p_chunk.tile([128, 512], F32, tag="zc", name="sb_r")
                nc.scalar.activation(out=sb_r, in_=pt_gr, func=AF.Sigmoid,
                                     bias=v_sb["b_r"])
                sb_z = p_chunk.tile([128, 512], F32, tag="zc", name="sb_z")
                nc.scalar.activation(out=sb_z, in_=pt_gz, func=AF.Sigmoid,
                                     bias=v_sb["b_z"])
                sb_hnb = p_chunk.tile([128, 512], F32, tag="nn", name="sb_hnb")
                nc.vector.tensor_scalar_add(sb_hnb, pt_gnh, v_sb["b_hn"])
                sb_rn = p_chunk.tile([128, 512], F32, tag="nn", name="sb_rn")
                nc.vector.tensor_mul(sb_rn, sb_r, sb_hnb)
                sb_np = p_chunk.tile([128, 512], F32, tag="nn", name="sb_np")
                nc.vector.tensor_add(sb_np, pt_gni, sb_rn)
                sb_nn = p_chunk.tile([128, 512], F32, tag="nn", name="sb_nn")
                nc.scalar.activation(out=sb_nn, in_=sb_np, func=AF.Tanh,
                                     bias=v_sb["b_in"])
                sb_hmn = p_chunk.tile([128, 512], F32, tag="nn", name="sb_hmn")
                nc.vector.tensor_sub(sb_hmn, sb_hT, sb_nn)
                sb_zh = p_chunk.tile([128, 512], F32, tag="nn", name="sb_zh")
                nc.vector.tensor_mul(sb_zh, sb_z, sb_hmn)
                nc.vector.tensor_add(seq_sl, sb_nn, sb_zh)
            return sb_srcT, sb_qp, sb_seqT

        def attn_phase(st, sb_srcT, sb_qp, sb_seqT):
            sb_mask = p_att.tile([128, 1024], BF16, tag="mask")
            nc.sync.dma_start(out=sb_mask, in_=t_mask[st])

            pt_sc = ps_sc.tile([128, 1024], F32, tag="sc")
            for ac in range(N_AC):
                base = ac * 1024
                for half in range(2):
                    nc.tensor.matmul(
                        pt_sc[:, half * 512:(half + 1) * 512],
                        sb_qp[ac],
                        sb_seqT[:, base + half * 512:base + (half + 1) * 512],
                        start=(ac == 0), stop=(ac == N_AC - 1))

            sb_scm = p_att.tile([128, 1024], F32, tag="scm")
            nc.vector.tensor_add(sb_scm, pt_sc, sb_mask)
            sb_attn = p_att.tile([128, 1024], F32, tag="attn")
            sb_sums = p_sm.tile([128, 1], F32, tag="sums")
            nc.scalar.activation(out=sb_attn, in_=sb_scm, func=AF.Exp,
                                 accum_out=sb_sums)
            sb_rec = p_stl.tile([128, 1], F32, tag="rec")
            sb_sum2 = p_sm.tile([128, 1], F32, tag="sums", name="sb_sum2")
            nc.vector.tensor_scalar_add(sb_sum2, sb_sums, 1e-30)
            nc.vector.reciprocal(sb_rec, sb_sum2)
            sb_attn_n = p_att.tile([128, 1024], F32, tag="attn_n")
            nc.vector.tensor_scalar_mul(sb_attn_n, sb_attn, sb_rec)

            pt_oaT = ps_sm.tile([128, 128], F32, tag="psmall", name="pt_oaT")
            for ac in range(N_AC):
                base = ac * 1024
                sb_vrm = p_att.tile([128, 8, 128], MMDT, tag="vrm", bufs=2)
                for half in range(2):
                    pt_v = ps_v.tile([128, 512], F32, tag="v")
                    for j in range(4):
                        sl = sb_seqT[:, base + half * 512 + j * 128:
                                     base + half * 512 + (j + 1) * 128]
                        nc.tensor.matmul(pt_v[:, j * 128:(j + 1) * 128],
                                         sl, w_sb["wvsT"],
                                         start=True, stop=True)
                    nc.vector.tensor_copy(out=sb_vrm[:, half * 4:(half + 1) * 4, :],
                                          in_=pt_v)
                pt_at = ps_v.tile([128, 256], F32, tag="v", name="pt_at")
                for j in range(8):
                    transpose(pt_at[:, j * 32:(j + 1) * 32],
                              sb_attn_n[32 * ac:32 * ac + 32, j * 128:(j + 1) * 128],
                              base=32 * ac, k=32)
                sb_at = p_sm.tile([128, 256], MMDT, tag="at")
                nc.vector.tensor_copy(out=sb_at, in_=pt_at)
                pt_oa = ps_sm.tile([32, 128], F32, tag="psmall", name="pt_oa")
                for j in range(8):
                    nc.tensor.matmul(pt_oa,
                                     sb_at[:, j * 32:(j + 1) * 32],
                                     sb_vrm[:, j, :],
                                     start=(j == 0), stop=(j == 7))
                sb_oa = p_sm.tile([32, 128], F32, tag="oa")
                nc.vector.tensor_copy(out=sb_oa, in_=pt_oa)
                transpose(pt_oaT[:, 32 * ac:32 * ac + 32], sb_oa, k=32)
            sb_oaT = p_sm.tile([128, 128], MMDT, tag="oaT")
            nc.vector.tensor_copy(out=sb_oaT, in_=pt_oaT)

            oaT_v = sb_oaT[:, :].rearrange("p (cg h) -> p h cg", h=2)
            pt_fc = ps_sm.tile([128, 128], F32, tag="psmall", name="pt_fc")
            nc.tensor.matmul(pt_fc[:, 0:64], w_sb["fcw0T"],
                             oaT_v[:, 0, :], start=True, stop=False)
            nc.tensor.matmul(pt_fc[:, 0:64], w_sb["fcw1T"],
                             oaT_v[:, 1, :], start=False, stop=True)

            sb_x1 = p_sm.tile([128, 64], F32, tag="x1")
            nc.vector.tensor_scalar_add(sb_x1, pt_fc[:, 0:64], v_sb["fc_b"])
            sb_x2 = p_sm.tile([128, 64], MMDT, tag="x2")
            nc.vector.tensor_add(sb_x2, sb_x1, sb_srcT)
            sb_sq = p_sm.tile([128, 64], MMDT, tag="sq")
            nc.scalar.activation(out=sb_sq, in_=sb_x2[:, :].bitcast(F32), func=AF.Square)
            pt_ln = ps_sm.tile([128, 128], F32, tag="psmall", name="pt_ln")
            nc.tensor.matmul(pt_ln[0:2, 0:64], ones2,
                             sb_x2, start=True, stop=True)
            nc.tensor.matmul(pt_ln[0:2, 64:128], ones2,
                             sb_sq, start=True, stop=True)
            sb_stats = p_sm.tile([1, 128], MMDT, tag="ln_stats")
            sb_mu = sb_stats[0:1, 0:64]
            nc.vector.tensor_scalar_mul(sb_mu, pt_ln[0:1, 0:64], 1.0 / 128.0)
            sb_ve = p_sm.tile([1, 64], F32, tag="ln_ve")
            sb_ex2 = p_sm.tile([1, 64], F32, tag="ln_ex2")
            nc.vector.tensor_scalar(sb_ex2, pt_ln[0:1, 64:128], 1.0 / 128.0, LN_EPS,
                                    op0=ALU.mult, op1=ALU.add)
            sb_musq = p_sm.tile([1, 64], F32, tag="ln_musq")
            nc.vector.tensor_mul(sb_musq, sb_mu, sb_mu)
            nc.vector.tensor_sub(sb_ve, sb_ex2, sb_musq)
            sb_y = p_sm.tile([1, 64], F32, tag="ln_y")
            sb_yi = p_sm.tile([1, 64], I32, tag="ln_yi")
            nc.vector.tensor_scalar(sb_yi, sb_ve[:, :].bitcast(I32), 1, None,
                                    op0=ALU.arith_shift_right)
            nc.vector.tensor_scalar(sb_y[:, :].bitcast(I32), sb_yi, -1, 0x5F3759DF,
                                    op0=ALU.mult, op1=ALU.add)
            for it in range(3):
                sb_t = p_sm.tile([1, 64], F32, tag="ln_t")
                nc.vector.tensor_mul(sb_t, sb_y, sb_y)
                sb_t2 = p_sm.tile([1, 64], F32, tag="ln_t2")
                nc.vector.tensor_mul(sb_t2, sb_t, sb_ve)
                sb_t3 = p_sm.tile([1, 64], F32, tag="ln_t3")
                nc.vector.tensor_scalar(sb_t3, sb_t2, -0.5, 1.5, op0=ALU.mult, op1=ALU.add)
                if it < 2:
                    sb_y2 = p_sm.tile([1, 64], F32, tag="ln_y2")
                else:
                    sb_y2 = sb_stats[0:1, 64:128]
                nc.vector.tensor_mul(sb_y2, sb_y, sb_t3)
                sb_y = sb_y2
            pt_bc = ps_sm.tile([128, 128], F32, tag="psmall", name="pt_bc")
            nc.tensor.matmul(pt_bc, ones_row, sb_stats,
                             start=True, stop=True)
            sb_xc = p_sm.tile([128, 64], F32, tag="xc")
            nc.vector.tensor_sub(sb_xc, sb_x2, pt_bc[:, 0:64])
            sb_xn0 = p_sm.tile([128, 64], F32, tag="xn0")
            nc.vector.tensor_mul(sb_xn0, sb_xc, pt_bc[:, 64:128])
            sb_xn = p_sm.tile([128, 64], MMDT, tag="xn")
            nc.vector.tensor_scalar(sb_xn, sb_xn0, v_sb["ln_g"], v_sb["ln_b"],
                                    op0=ALU.mult, op1=ALU.add)

            pt_h1 = ps_sm.tile([128, 128], F32, tag="psmall", name="pt_h1")
            nc.tensor.matmul(pt_h1[:, 0:64], w_sb["m1aT"],
                             sb_xn, start=True, stop=False)
            nc.tensor.matmul(pt_h1[:, 0:64], w_sb["m1bT"],
                             sb_srcT, start=False, stop=True)
            sb_h1 = p_sm.tile([128, 64], MMDT, tag="h1")
            nc.scalar.activation(out=sb_h1, in_=pt_h1[:, 0:64], func=AF.Relu,
                                 bias=v_sb["m1b"])
            pt_z = ps_sm.tile([128, 128], F32, tag="psmall", name="pt_z")
            nc.tensor.matmul(pt_z[:, 0:64], w_sb["m2T"],
                             sb_h1, start=True, stop=True)
            sb_zb = p_sm.tile([128, 64], F32, tag="zb")
            nc.vector.tensor_scalar_add(sb_zb, pt_z[:, 0:64], v_sb["m2b"])
            pt_zr = ps_sm.tile([128, 128], F32, tag="psmall", name="pt_zr")
            transpose(pt_zr[0:64, :], sb_zb)
            sb_zout = p_sm.tile([64, 128], F32, tag="zout")
            nc.scalar.copy(out=sb_zout, in_=pt_zr[0:64, :])
            nc.sync.dma_start(out=t_out[st * 64:(st + 1) * 64, :], in_=sb_zout)

        for grp in range(N_ST // 4):
            sts = range(grp * 4, (grp + 1) * 4)
            carry = [gru_phase(st) for st in sts]
            for st, c in zip(sts, carry):
                attn_phase(st, *c)

    nc.finalize()
    return nc


# ----------------------------------------------------------------------------
# Host side
# ----------------------------------------------------------------------------

def _prep_inputs(inputs, general):
    """Build per-core input maps (numpy) from full-size inputs."""
    f32 = np.float32
    bf16 = ml_dtypes.bfloat16
    src = np.ascontiguousarray(np.asarray(inputs["src"], f32))
    ngh = np.ascontiguousarray(np.asarray(inputs["ngh_feat"], f32))
    mask = np.asarray(inputs["mask"]).astype(bool)
    w_qs = np.asarray(inputs["w_qs"], f32)
    w_ks = np.asarray(inputs["w_ks"], f32)
    w_vs = np.asarray(inputs["w_vs"], f32)
    fc_w = np.asarray(inputs["fc_w"], f32)
    w_ih = np.asarray(inputs["gru_w_ih"], f32)
    m_fc1 = np.asarray(inputs["m_fc1_w"], f32)
    m_fc2 = np.asarray(inputs["m_fc2_w"], f32)

    wdt = f32 if general else bf16
    # fast path folds seq' = -2*seq into w_q (scores) and the fc fold (values)
    qscale = 1.0 if general else -0.5
    # fast path: z-gate tanh(gz/2) folded into wihzT; W_v folded into fc:
    #   fc(attn@v) = (attn@seq') @ (fc_w_h @ W_v_h)^T * (-0.5) per head
    if general:
        fcw0 = fc_w.T * (np.arange(128) < 64)[:, None].astype(f32)
        fcw1 = fc_w.T * (np.arange(128) >= 64)[:, None].astype(f32)
        zsc = 1.0
    else:
        fcw0 = (fc_w[:, 0:64] @ w_vs[0:64, :]).T * -0.5
        fcw1 = (fc_w[:, 64:128] @ w_vs[64:128, :]).T * -0.5
        zsc = 0.5
    wmats = {
        "wqT": (w_qs / TEMP).T * qscale,
        "wks": w_ks,
        "wihzT": w_ih[128:256].T * zsc,
        "wihnT": w_ih[256:384].T,
        "fcw0T": fcw0,
        "fcw1T": fcw1,
        "m1aT": m_fc1[:, :128].T,
        "m1bT": m_fc1[:, 128:].T,
        "m2T": m_fc2.T,
    }
    vvecs = {
        "fc_b": np.asarray(inputs["fc_b"], f32).reshape(128, 1),
        "ln_g": np.asarray(inputs["ln_g"], f32).reshape(128, 1),
        "ln_b": np.asarray(inputs["ln_b"], f32).reshape(128, 1),
        "m1b": np.asarray(inputs["m_fc1_b"], f32).reshape(128, 1),
        "m2b": np.asarray(inputs["m_fc2_b"], f32).reshape(128, 1),
    }
    wnames = ["wqT", "wks", "wihzT", "wihnT",
              "fcw0T", "fcw1T", "m1aT", "m1bT", "m2T"]
    vnames = ["fc_b", "ln_g", "ln_b", "m1b", "m2b"]
    com = {"eye": np.eye(128, dtype=f32), "onesrow": np.ones((1, 128), wdt)}
    if general:
        for n, w in wmats.items():
            com[n] = np.ascontiguousarray(w).astype(f32)
        for n, v in vvecs.items():
            com[n] = v
        com["ones2"] = np.concatenate(
            [np.ones((128, 1), f32), np.zeros((128, 1), f32)], 1)
        com["zeros128"] = np.zeros((128, 128), f32)
        com["wvsT"] = np.ascontiguousarray(w_vs.T).astype(f32)
    else:
        ones2pad = np.zeros((128, 4), f32)
        ones2pad[:, 0] = 1.0
        wpack = np.concatenate(
            [wmats[n] for n in wnames] + [np.eye(128, dtype=f32), ones2pad],
            axis=1)
        com["wpack"] = np.ascontiguousarray(wpack).astype(bf16)
        com["vpack"] = np.ascontiguousarray(
            np.concatenate([vvecs[n] for n in vnames], axis=1))
        w_hh = np.asarray(inputs["gru_w_hh"], f32)
        b_ih = np.asarray(inputs["gru_b_ih"], f32)
        b_hh = np.asarray(inputs["gru_b_hh"], f32)
        com.update({
            "wihrT": np.ascontiguousarray(w_ih[0:128].T),
            "whhrT": np.ascontiguousarray(w_hh[0:128].T),
            "whhzT": np.ascontiguousarray(w_hh[128:256].T),
            "whhnT": np.ascontiguousarray(w_hh[256:384].T),
            "b_r": (b_ih[0:128] + b_hh[0:128]).reshape(128, 1).astype(f32),
            "b_z": (b_ih[128:256] + b_hh[128:256]).reshape(128, 1).astype(f32),
            "b_in": b_ih[256:384].reshape(128, 1).astype(f32),
            "b_hn": b_hh[256:384].reshape(128, 1).astype(f32),
        })

    # additive mask, per core: [N_ST, 128(=32ac+2g+h), 1024] (bf16)
    m3 = mask.reshape(N_CORES, B_CORE, N_SRC, NN)  # [core, b, s, n]
    st_i = np.arange(N_ST)
    cc_i = np.arange(4)
    g_i = np.arange(16)
    b_idx = 2 * st_i[:, None] + cc_i[None, :] // 2          # [st, cc]
    s_idx = (cc_i[:, None] % 2) * 16 + g_i[None, :]         # [cc, g]
    # fast path: score column c of any ac-block maps to within-block row
    # perm[c] = ((c%128)//8)*64 + (c%8)*8 + c//128  (4KB-segment x layout)
    c_arr = np.arange(1024)
    col_perm = ((c_arr % 128) // 8) * 64 + (c_arr % 8) * 8 + c_arr // 128
    maskfull_cores = []
    for core in range(N_CORES):
        msel = m3[core][b_idx[:, :, None], s_idx[None, :, :]]   # [st, cc, g, 64]
        vals = np.where(msel, f32(NEG_INF), f32(0.0))           # [st, cc, g, 64]
        out = np.full((N_ST, 4, 16, 2, 16, 64), NEG_INF, f32)
        out[:, :, g_i, :, g_i, :] = vals.transpose(2, 0, 1, 3)[:, :, :, None, :]
        out = out.reshape(N_ST, 128, 1024)
        if not general:
            out = out[:, :, col_perm]
        maskfull_cores.append(np.ascontiguousarray(out).astype(bf16))

    in_maps = []
    hid = None
    if general:
        hid = np.ascontiguousarray(np.asarray(inputs["hidden"], f32))
    for core in range(N_CORES):
        m = dict(com)
        m["ngh"] = ngh[core * ROWS:(core + 1) * ROWS]
        m["srcf"] = src[core * B_CORE:(core + 1) * B_CORE].reshape(B_CORE * N_SRC, D)
        m["maskfull"] = maskfull_cores[core]
        if general:
            m["hid"] = hid[core * ROWS:(core + 1) * ROWS]
        in_maps.append(m)
    return in_maps


def _get_program(general, tune=None):
    key = (general, tuple(sorted((tune or {}).items())))
    if key not in _PROG_CACHE:
        if general:
            _PROG_CACHE[key] = build_program_v1(True)
        else:
            _PROG_CACHE[key] = build_program_fast(tune)
    return _PROG_CACHE[key]


def _is_fast_path(inputs):
    if np.asarray(inputs["gru_b_ih"]).any() or np.asarray(inputs["gru_b_hh"]).any():
        return False
    return not np.asarray(inputs["hidden"]).any()


def run(inputs, trace=False, force_general=None, tune=None):
    if tune is None and os.environ.get("K_TUNE"):
        tune = dict(kv.split("=") for kv in os.environ["K_TUNE"].split(","))
        tune = {k: int(v) for k, v in tune.items()}
    from concourse.bass_utils import run_bass_kernel_spmd
    general = (not _is_fast_path(inputs)) if force_general is None else force_general
    nc = _get_program(general, tune)
    in_maps = _prep_inputs(inputs, general)
    res = run_bass_kernel_spmd(nc, in_maps, list(range(N_CORES)), trace=trace)
    z = np.stack([r["z"] for r in res.results], axis=0)  # [8, 1024, 128]
    out = z.reshape(N_CORES, B_CORE, N_SRC, D).reshape(B, N_SRC, D).astype(np.float32)
    return out, res


def kernel(**inputs) -> np.ndarray:
    out, _ = run(inputs, trace=False)
    return out
